# revision 28
# baseline (speedup 1.0000x reference)
"""Self-contained TRN2 Bass kernel for the 16-head MHA problem.

kernel(**inputs) takes FULL inputs (x [4,2048,1024], Wq/Wk/Wv [16,1024,64],
Wo [1024,1024], bo [1024]) and returns the FULL output [4,2048,1024] f32.

Sharding over 8 NeuronCores: core c handles batch b = c//2 and head group
g = c%2 (8 of 16 heads) — tensor parallel over heads with the output
projection's input dim sharded; the 2-way partial-sum reduce per batch and
the bias add happen host-side on the gathered results.
"""
import sys

for _p in ("/opt/trn_rl_repo",):
    if _p not in sys.path:
        sys.path.insert(0, _p)

import numpy as np
import concourse.bass as bass
import concourse.mybir as mybir
from concourse import bacc
from concourse.bass import ts, ds
from concourse.tile import TileContext
from concourse.vector_clock import ScopedClock
from concourse import bass_utils

F32 = mybir.dt.float32
BF16 = mybir.dt.bfloat16
AF = mybir.ActivationFunctionType

NUM_HEADS = 16
EMB = 1024
HEAD = 64
SEQ = 2048
BATCH = 4
N_CORES = 8


class TC(TileContext):
    """TileContext whose final drain splits its sem waits across SP NOPs —
    the CTRL instruction encoding holds only one wait and this env's Tile
    puts the whole global clock on the tail drain."""

    def _drain_and_barrier(self, tick_clock, wait_clock):
        nc = self.nc
        dummy = nc.sync.nop(nofuse=True)
        wait_clock.add_sem_waits(dummy.ins, ScopedClock({None: tick_clock.global_clock}))
        si = dummy.ins.sync_info
        waits = list(si.on_wait) if si is not None else []
        if len(waits) > 1:
            si.on_wait = waits[:1]
            sem_by_name = {h.name: h for h in self.sems.allocated().values()}
            for w in waits[1:]:
                nop = nc.sync.nop(nofuse=True)
                nop._wait_ge(sem_by_name[w.ant_name], w.wait_value)
        nc.sync.drain()
        nc.all_engine_barrier()
        popped = nc._tile_sem_poison_stack.pop()
        assert popped is self._sem_poison
        nc.clear_and_free_semaphores(list(self.sems.allocated().values()))
        nc.all_engine_barrier()


def build_mha_nc(S=SEQ, E=EMB, D=HEAD, H=NUM_HEADS // 2):
    """Single-core SPMD program; H = heads per core (pair-packed).

    Fully transposed formulation:
      xT (PE-transposed once), qT/kT per pair (q pre-scaled by 1/sqrt(D)),
      scoresT = kT-slice.T @ qT-slice (K=64 row groups, both heads of a
      pair into one 2-bank PSUM tile),
      attnT = exp(scoresT) (scores bounded, no max subtraction; one ACT op
      and one causal affine_select per pair-tile),
      ctxT+denominator = [v|1].T @ attnT accumulated over k chunks,
      reciprocal of the denominator via DMA-reshape to [128, x] so all DVE
      lanes work, broadcast along d via a K=1 ones matmul,
      out = concatT-chunk.T @ WoT-chunk accumulated over head-dim chunks.

    Scheduling structure: q/k projections for pair p+1 are interleaved
    into pair p's attention groups to fill PE gaps; ce accumulators are
    tagged by q-chunk parity so a group's normalization tail overlaps the
    next group's matmuls.
    """
    P = 128
    EC = E // P
    NQ = 512
    J = S // NQ
    KK = S // P
    NP = H // 2
    HD = H * D
    HC = HD // P
    NE = min(512, E)
    JE = E // NE
    RQ = NQ // 64          # reshape width for the two-row reciprocal trick

    nc = bacc.Bacc("TRN2", target_bir_lowering=False, debug=False)
    # host-prearranged bf16 inputs (cast + transpose/layout done in kernel())
    xT_in = nc.dram_tensor("xT_in", [P, EC, S], BF16, kind="ExternalInput")
    wq_in = nc.dram_tensor("wq_in", [P, NP, EC, 2 * D], BF16, kind="ExternalInput")
    wk_in = nc.dram_tensor("wk_in", [P, NP, EC, 2 * D], BF16, kind="ExternalInput")
    wv_in = nc.dram_tensor("wv_in", [P, EC, H, D], BF16, kind="ExternalInput")
    wo_in = nc.dram_tensor("wo_in", [P, HC, E], BF16, kind="ExternalInput")
    out_p = nc.dram_tensor("out_p", [S, E], F32, kind="ExternalOutput")

    scale = D ** -0.5

    with TC(nc) as tc:
        with (
            tc.tile_pool(name="const", bufs=1) as cpool,
            tc.tile_pool(name="persist", bufs=1) as pers,
            tc.tile_pool(name="stage", bufs=3) as stg,
            tc.tile_pool(name="attn", bufs=6) as apool,
            tc.tile_pool(name="small", bufs=3) as spool,
            tc.tile_pool(name="psS", bufs=4, space="PSUM") as psS,
            tc.tile_pool(name="psC", bufs=1, space="PSUM") as psC,
        ):
            ones_col = cpool.tile([1, D], mybir.dt.float16, tag="ones")
            nc.gpsimd.memset(ones_col[:], 1.0)

            xT = pers.tile([P, EC, S], BF16, tag="xT")
            qT = pers.tile([P, NP, S], BF16, tag="qT")
            kT = pers.tile([P, NP, S], BF16, tag="kT")
            v_pad = pers.tile([P, KK, H, D + 1], BF16, tag="vp")
            woT = pers.tile([P, HC, E], BF16, tag="woT")
            concatT = pers.tile([P, NP, S], BF16, tag="concT")
            wq_bf = pers.tile([P, NP, EC, 2 * D], BF16, tag="wq")
            wk_bf = pers.tile([P, NP, EC, 2 * D], BF16, tag="wk")
            wv_bf = pers.tile([P, EC, H, D], BF16, tag="wv")

            nc.gpsimd.memset(v_pad[:, :, :, D:D + 1], 1.0)

            # ---- Phase A: stream host-prearranged bf16 xT + weights, with
            # v / pair-0+1 q/k projections and pair-0 attention fused in ----
            def emit_v(sc):
                acc = psS.tile([P, HD], F32, tag="sc2", name=f"vacc_{sc}")
                for ec in range(EC):
                    nc.tensor.matmul(
                        acc[:],
                        xT[:, ec, ts(sc, P)],
                        wv_bf[:, ec, :, :].rearrange("p h d -> p (h d)"),
                        start=(ec == 0), stop=(ec == EC - 1),
                    )
                nc.vector.tensor_copy(
                    v_pad[:, sc, :, 0:D],
                    acc[:].rearrange("p (h d) -> p h d", d=D),
                )

            def emit_qk(p2, sc):
                for w_sb, dst, sc_mul in ((wq_bf, qT, scale), (wk_bf, kT, 1.0)):
                    acc = psS.tile(
                        [P, NQ], F32, tag="sc2",
                        name=f"qk_{p2}_{sc}_{0 if dst is qT else 1}",
                    )
                    for ec in range(EC):
                        nc.tensor.matmul(
                            acc[:],
                            w_sb[:, p2, ec, :],
                            xT[:, ec, ts(sc, NQ)],
                            start=(ec == 0), stop=(ec == EC - 1),
                        )
                    if sc_mul != 1.0:
                        nc.vector.tensor_scalar_mul(
                            dst[:, p2, ts(sc, NQ)], acc[:], sc_mul
                        )
                    else:
                        nc.vector.tensor_copy(dst[:, p2, ts(sc, NQ)], acc[:])

            def emit_out_chunks(j):
                for sc in range(4 * j, 4 * j + 4):
                    for n in range(JE):
                        acc = psS.tile([P, NE], F32, tag="sc2",
                                       name=f"oacc_{sc}_{n}")
                        for hc in range(HC):
                            nc.tensor.matmul(
                                acc[:],
                                concatT[:, hc, ts(sc, P)],
                                woT[:, hc, ts(n, NE)],
                                start=(hc == 0), stop=(hc == HC - 1),
                            )
                        ot = stg.tile([P, NE], F32, tag="ostg")
                        nc.vector.tensor_copy(ot[:], acc[:])
                        nc.sync.dma_start(out_p[ts(sc, P), ts(n, NE)], ot[:])

            def emit_normalize(p2, j, ce):
                dens = [
                    spool.tile([1, NQ], F32, tag=f"den{hh}",
                               name=f"den{hh}_{p2}_{j}")
                    for hh in range(2)
                ]
                for hh in range(2):
                    nc.vector.tensor_copy(dens[hh][:], ce[hh][ds(D, 1), :])
                den_rs = spool.tile([P, RQ], F32, tag="denrs")
                for hh in range(2):
                    nc.sync.dma_start(
                        den_rs[ds(hh * (P // 2), P // 2), :], dens[hh][:]
                    )
                rc = spool.tile([P, RQ], F32, tag="rc")
                nc.vector.reciprocal(rc[:], den_rs[:])
                rch = spool.tile([P, RQ], mybir.dt.float16, tag="rch")
                nc.vector.tensor_copy(rch[:], rc[:])
                recips = [
                    spool.tile([1, NQ], mybir.dt.float16, tag=f"recip{hh}",
                               name=f"recip{hh}_{p2}_{j}")
                    for hh in range(2)
                ]
                for hh in range(2):
                    nc.sync.dma_start(
                        recips[hh][:], rch[ds(hh * (P // 2), P // 2), :]
                    )
                for hh in range(2):
                    bc = psS.tile([D, NQ], F32, tag="sc2", name=f"bc_{p2}_{j}_{hh}")
                    nc.tensor.matmul(
                        bc[:], ones_col[:], recips[hh][:],
                        start=True, stop=True,
                    )
                    bc_sb = spool.tile([D, NQ], F32, tag="bcsb")
                    nc.vector.tensor_copy(bc_sb[:], bc[:])
                    nc.vector.tensor_mul(
                        concatT[ds(hh * D, D), p2, ts(j, NQ)],
                        ce[hh][0:D, :], bc_sb[:],
                    )

            pending = [None]

            def flush_pending():
                if pending[0] is not None:
                    p2x, jx, cex = pending[0]
                    emit_normalize(p2x, jx, cex)
                    if p2x == NP - 1:
                        emit_out_chunks(jx)
                    pending[0] = None

            def emit_group(p2, j):
                h0, h1 = 2 * p2, 2 * p2 + 1
                n_kk = min(KK, 4 * j + 4)
                ce = [
                    psC.tile([D + 1, NQ], F32, tag=f"ce{hh}{j % 2}",
                             name=f"ce{hh}_{p2}_{j}")
                    for hh in range(2)
                ]
                for i in range(n_kk):
                    t = i - 4 * j  # >= 0 -> diagonal (partial) tile
                    q0 = P * t if t > 0 else 0
                    nq = NQ - q0
                    for hh, hloc in enumerate((h0, h1)):
                        sps = psS.tile([P, NQ], F32, tag="sc2",
                                       name=f"s2_{p2}_{j}_{i}_{hh}")
                        nc.tensor.matmul(
                            sps[:, 0:nq],
                            kT[ds(hh * D, D), p2, ts(i, P)],
                            qT[ds(hh * D, D), p2, ds(j * NQ + q0, nq)],
                            start=True, stop=True,
                        )
                        at = apool.tile([P, NQ], BF16, tag="at")
                        nc.scalar.activation(at[:, 0:nq], sps[:, 0:nq], AF.Exp)
                        if t >= 0:
                            nc.gpsimd.affine_select(
                                out=at[:, 0:nq], in_=at[:, 0:nq],
                                compare_op=mybir.AluOpType.is_ge,
                                fill=0.0, base=P * t - q0,
                                pattern=[[1, nq]], channel_multiplier=-1,
                            )
                        nc.tensor.matmul(
                            ce[hh][0:D + 1, ds(q0, nq)],
                            v_pad[:, i, hloc, :],
                            at[:, 0:nq],
                            start=(i == 0), stop=(i == n_kk - 1),
                        )
                    if i == min(3, n_kk - 1):
                        flush_pending()
                flush_pending()
                pending[0] = (p2, j, ce)

            nc.sync.dma_start(xT[:, :, ts(0, NQ)], xT_in[:, :, ts(0, NQ)])
            nc.sync.dma_start(wv_bf[:], wv_in[:])
            nc.sync.dma_start(wq_bf[:], wq_in[:])
            nc.sync.dma_start(wk_bf[:], wk_in[:])
            for c4 in range(J):
                if c4 + 1 < J:
                    nc.sync.dma_start(
                        xT[:, :, ts(c4 + 1, NQ)], xT_in[:, :, ts(c4 + 1, NQ)]
                    )
                for sc in range(4 * c4, 4 * c4 + 4):
                    emit_v(sc)
                emit_qk(0, c4)
                emit_qk(1, c4)
                if c4 == 0:
                    nc.sync.dma_start(woT[:], wo_in[:])
                if c4 > 0:
                    emit_group(0, c4 - 1)

            # ---- remaining attention groups (pair 0 groups 0-1 were emitted
            # inside the x loop); next pair's q/k interleaved ----
            for p2 in range(NP):
                for j in range(J):
                    if p2 == 0 and j < 3:
                        continue
                    emit_group(p2, j)
                    if p2 >= 1 and p2 + 1 < NP:
                        emit_qk(p2 + 1, j)
            flush_pending()

    nc.finalize()
    return nc


_NC_CACHE = {}


def _get_nc():
    key = "mha"
    if key not in _NC_CACHE:
        _NC_CACHE[key] = build_mha_nc()
    return _NC_CACHE[key]


def _arr_xT(xb, bf16):
    # [S, E] f32 -> [P, EC, S] bf16 with xT[p, ec, s] = x[s, ec*128+p]
    P, S, E = 128, xb.shape[0], xb.shape[1]
    xt = xb.astype(bf16).T.reshape(E // P, P, S)
    return np.ascontiguousarray(xt.transpose(1, 0, 2))


def _arr_wqk(w, bf16):
    # [H, E, D] -> [P, NP, EC, 2*D] pair-packed lhsT layout
    H, E, D = w.shape
    P = 128
    v = w.astype(bf16).reshape(H // 2, 2, E // P, P, D)
    v = v.transpose(3, 0, 2, 1, 4)  # [P, NP, EC, 2, D]
    return np.ascontiguousarray(v.reshape(P, H // 2, E // P, 2 * D))


def _arr_wv(w, bf16):
    # [H, E, D] -> [P, EC, H, D]
    H, E, D = w.shape
    P = 128
    v = w.astype(bf16).reshape(H, E // P, P, D)
    return np.ascontiguousarray(v.transpose(2, 1, 0, 3))


def _arr_wo(w, bf16):
    # [E, HD] -> [P, HC, E] with woT[p, hc, e] = Wo[e, hc*128+p]
    E, HD = w.shape
    P = 128
    v = w.astype(bf16).T.reshape(HD // P, P, E)
    return np.ascontiguousarray(v.transpose(1, 0, 2))


def kernel(x, Wq, Wk, Wv, Wo, bo, _runner_kwargs=None):
    import ml_dtypes
    bf16 = ml_dtypes.bfloat16
    x = np.asarray(x, dtype=np.float32)
    Wq = np.asarray(Wq, dtype=np.float32)
    Wk = np.asarray(Wk, dtype=np.float32)
    Wv = np.asarray(Wv, dtype=np.float32)
    Wo = np.asarray(Wo, dtype=np.float32)
    bo = np.asarray(bo, dtype=np.float32)

    HPC = NUM_HEADS // 2  # heads per core
    HDS = HPC * HEAD      # concat-dim slice per core

    nc = _get_nc()
    xTs = [_arr_xT(x[b], bf16) for b in range(BATCH)]
    in_maps = []
    for c in range(N_CORES):
        b, g = c // 2, c % 2
        hs = slice(g * HPC, (g + 1) * HPC)
        in_maps.append({
            "xT_in": xTs[b],
            "wq_in": _arr_wqk(Wq[hs], bf16),
            "wk_in": _arr_wqk(Wk[hs], bf16),
            "wv_in": _arr_wv(Wv[hs], bf16),
            "wo_in": _arr_wo(Wo[:, g * HDS:(g + 1) * HDS], bf16),
        })

    kw = dict(_runner_kwargs or {})
    res = bass_utils.run_bass_kernel_spmd(
        nc, in_maps, core_ids=list(range(N_CORES)), **kw
    )

    out = np.empty((BATCH, SEQ, EMB), dtype=np.float32)
    for b in range(BATCH):
        out[b] = res.results[2 * b]["out_p"] + res.results[2 * b + 1]["out_p"] + bo
    if kw.get("trace"):
        kernel.last_results = res
    return out


# revision 30
# speedup vs baseline: 1.1357x; 1.1357x over previous
"""Self-contained TRN2 Bass kernel for the 16-head MHA problem.

kernel(**inputs) takes FULL inputs (x [4,2048,1024], Wq/Wk/Wv [16,1024,64],
Wo [1024,1024], bo [1024]) and returns the FULL output [4,2048,1024] f32.

Sharding over 8 NeuronCores: core c handles batch b = c//2 and head group
g = c%2 (8 of 16 heads) — tensor parallel over heads with the output
projection's input dim sharded; the 2-way partial-sum reduce per batch and
the bias add happen host-side on the gathered results.
"""
import sys

for _p in ("/opt/trn_rl_repo",):
    if _p not in sys.path:
        sys.path.insert(0, _p)

import numpy as np
import concourse.bass as bass
import concourse.mybir as mybir
from concourse import bacc
from concourse.bass import ts, ds
from concourse.masks import make_identity
from concourse.tile import TileContext
from concourse.vector_clock import ScopedClock
from concourse import bass_utils

F32 = mybir.dt.float32
BF16 = mybir.dt.bfloat16
AF = mybir.ActivationFunctionType

NUM_HEADS = 16
EMB = 1024
HEAD = 64
SEQ = 2048
BATCH = 4
N_CORES = 8


class TC(TileContext):
    """TileContext whose final drain splits its sem waits across SP NOPs —
    the CTRL instruction encoding holds only one wait and this env's Tile
    puts the whole global clock on the tail drain."""

    def _drain_and_barrier(self, tick_clock, wait_clock):
        nc = self.nc
        dummy = nc.sync.nop(nofuse=True)
        wait_clock.add_sem_waits(dummy.ins, ScopedClock({None: tick_clock.global_clock}))
        si = dummy.ins.sync_info
        waits = list(si.on_wait) if si is not None else []
        if len(waits) > 1:
            si.on_wait = waits[:1]
            sem_by_name = {h.name: h for h in self.sems.allocated().values()}
            for w in waits[1:]:
                nop = nc.sync.nop(nofuse=True)
                nop._wait_ge(sem_by_name[w.ant_name], w.wait_value)
        nc.sync.drain()
        nc.all_engine_barrier()
        popped = nc._tile_sem_poison_stack.pop()
        assert popped is self._sem_poison
        nc.clear_and_free_semaphores(list(self.sems.allocated().values()))
        nc.all_engine_barrier()


def build_mha_nc(S=SEQ, E=EMB, D=HEAD, H=NUM_HEADS // 2):
    """Single-core SPMD program; H = heads per core (pair-packed).

    Fully transposed formulation:
      xT (PE-transposed once), qT/kT per pair (q pre-scaled by 1/sqrt(D)),
      scoresT = kT-slice.T @ qT-slice (K=64 row groups, both heads of a
      pair into one 2-bank PSUM tile),
      attnT = exp(scoresT) (scores bounded, no max subtraction; one ACT op
      and one causal affine_select per pair-tile),
      ctxT+denominator = [v|1].T @ attnT accumulated over k chunks,
      reciprocal of the denominator via DMA-reshape to [128, x] so all DVE
      lanes work, broadcast along d via a K=1 ones matmul,
      out = concatT-chunk.T @ WoT-chunk accumulated over head-dim chunks.

    Scheduling structure: q/k projections for pair p+1 are interleaved
    into pair p's attention groups to fill PE gaps; ce accumulators are
    tagged by q-chunk parity so a group's normalization tail overlaps the
    next group's matmuls.
    """
    P = 128
    EC = E // P
    NQ = 512
    J = S // NQ
    KK = S // P
    NP = H // 2
    HD = H * D
    HC = HD // P
    NE = min(512, E)
    JE = E // NE
    RQ = NQ // 64          # reshape width for the two-row reciprocal trick

    nc = bacc.Bacc("TRN2", target_bir_lowering=False, debug=False)
    # host pre-casts x to bf16 and pre-arranges the (small) weights into
    # their on-chip layouts, halving the startup DMA bytes
    x_b = nc.dram_tensor("x_b", [S, E], BF16, kind="ExternalInput")
    wq_in = nc.dram_tensor("wq_in", [P, NP, EC, 2 * D], BF16, kind="ExternalInput")
    wk_in = nc.dram_tensor("wk_in", [P, NP, EC, 2 * D], BF16, kind="ExternalInput")
    wv_in = nc.dram_tensor("wv_in", [P, EC, H, D], BF16, kind="ExternalInput")
    wo_in = nc.dram_tensor("wo_in", [P, HC, E], BF16, kind="ExternalInput")
    out_p = nc.dram_tensor("out_p", [S, E], F32, kind="ExternalOutput")

    scale = D ** -0.5

    with TC(nc) as tc:
        with (
            tc.tile_pool(name="const", bufs=1) as cpool,
            tc.tile_pool(name="persist", bufs=1) as pers,
            tc.tile_pool(name="stage", bufs=3) as stg,
            tc.tile_pool(name="attn", bufs=6) as apool,
            tc.tile_pool(name="small", bufs=3) as spool,
            tc.tile_pool(name="psS", bufs=4, space="PSUM") as psS,
            tc.tile_pool(name="psC", bufs=1, space="PSUM") as psC,
        ):
            identb = cpool.tile([P, P], BF16, tag="identb")
            make_identity(nc, identb[:])
            ones_col = cpool.tile([1, D], mybir.dt.float16, tag="ones")
            nc.gpsimd.memset(ones_col[:], 1.0)

            xT = pers.tile([P, EC, S], BF16, tag="xT")
            qT = pers.tile([P, NP, S], BF16, tag="qT")
            kT = pers.tile([P, NP, S], BF16, tag="kT")
            v_pad = pers.tile([P, KK, H, D + 1], BF16, tag="vp")
            woT = pers.tile([P, HC, E], BF16, tag="woT")
            concatT = pers.tile([P, NP, S], BF16, tag="concT")
            wq_bf = pers.tile([P, NP, EC, 2 * D], BF16, tag="wq")
            wk_bf = pers.tile([P, NP, EC, 2 * D], BF16, tag="wk")
            wv_bf = pers.tile([P, EC, H, D], BF16, tag="wv")

            nc.gpsimd.memset(v_pad[:, :, :, D:D + 1], 1.0)

            # ---- Phase A: stream bf16 x with transposes, v and pair-0 q/k
            # fused in; weights land via single pre-arranged DMAs ----
            xf_pre = {}
            for sc in range(2):
                xf = stg.tile([P, E], BF16, tag="xstg", name=f"xf_pre{sc}")
                nc.sync.dma_start(xf[:], x_b[ts(sc, P), :])
                xf_pre[sc] = xf
            nc.sync.dma_start(wv_bf[:], wv_in[:])
            nc.sync.dma_start(wq_bf[:], wq_in[:])
            nc.sync.dma_start(wk_bf[:], wk_in[:])

            def emit_qk(p2, sc):
                # sc indexes NQ-wide chunks (two per call site index)
                for w_sb, dst, sc_mul in ((wq_bf, qT, scale), (wk_bf, kT, 1.0)):
                    acc = psS.tile(
                        [P, NQ], F32, tag="sc2",
                        name=f"qk_{p2}_{sc}_{0 if dst is qT else 1}",
                    )
                    for ec in range(EC):
                        nc.tensor.matmul(
                            acc[:],
                            w_sb[:, p2, ec, :],
                            xT[:, ec, ts(sc, NQ)],
                            start=(ec == 0), stop=(ec == EC - 1),
                        )
                    if sc_mul != 1.0:
                        nc.vector.tensor_scalar_mul(
                            dst[:, p2, ts(sc, NQ)], acc[:], sc_mul
                        )
                    else:
                        nc.vector.tensor_copy(dst[:, p2, ts(sc, NQ)], acc[:])

            def emit_v(sc):
                acc = psS.tile([P, HD], F32, tag="sc2", name=f"vacc_{sc}")
                for ec in range(EC):
                    nc.tensor.matmul(
                        acc[:],
                        xT[:, ec, ts(sc, P)],
                        wv_bf[:, ec, :, :].rearrange("p h d -> p (h d)"),
                        start=(ec == 0), stop=(ec == EC - 1),
                    )
                nc.vector.tensor_copy(
                    v_pad[:, sc, :, 0:D],
                    acc[:].rearrange("p (h d) -> p h d", d=D),
                )

            def emit_out_chunks(j):
                for sc in range(4 * j, 4 * j + 4):
                    for n in range(JE):
                        acc = psS.tile([P, NE], F32, tag="sc2",
                                       name=f"oacc_{sc}_{n}")
                        for hc in range(HC):
                            nc.tensor.matmul(
                                acc[:],
                                concatT[:, hc, ts(sc, P)],
                                woT[:, hc, ts(n, NE)],
                                start=(hc == 0), stop=(hc == HC - 1),
                            )
                        ot = stg.tile([P, NE], F32, tag="ostg")
                        nc.vector.tensor_copy(ot[:], acc[:])
                        nc.sync.dma_start(out_p[ts(sc, P), ts(n, NE)], ot[:])

            def emit_normalize(p2, j, ce):
                dens = [
                    spool.tile([1, NQ], F32, tag=f"den{hh}",
                               name=f"den{hh}_{p2}_{j}")
                    for hh in range(2)
                ]
                for hh in range(2):
                    nc.vector.tensor_copy(dens[hh][:], ce[hh][ds(D, 1), :])
                den_rs = spool.tile([P, RQ], F32, tag="denrs")
                for hh in range(2):
                    nc.sync.dma_start(
                        den_rs[ds(hh * (P // 2), P // 2), :], dens[hh][:]
                    )
                rc = spool.tile([P, RQ], F32, tag="rc")
                nc.vector.reciprocal(rc[:], den_rs[:])
                rch = spool.tile([P, RQ], mybir.dt.float16, tag="rch")
                nc.vector.tensor_copy(rch[:], rc[:])
                recips = [
                    spool.tile([1, NQ], mybir.dt.float16, tag=f"recip{hh}",
                               name=f"recip{hh}_{p2}_{j}")
                    for hh in range(2)
                ]
                for hh in range(2):
                    nc.sync.dma_start(
                        recips[hh][:], rch[ds(hh * (P // 2), P // 2), :]
                    )
                for hh in range(2):
                    bc = psS.tile([D, NQ], F32, tag="sc2", name=f"bc_{p2}_{j}_{hh}")
                    nc.tensor.matmul(
                        bc[:], ones_col[:], recips[hh][:],
                        start=True, stop=True,
                    )
                    bc_sb = spool.tile([D, NQ], F32, tag="bcsb")
                    nc.vector.tensor_copy(bc_sb[:], bc[:])
                    nc.vector.tensor_mul(
                        concatT[ds(hh * D, D), p2, ts(j, NQ)],
                        ce[hh][0:D, :], bc_sb[:],
                    )

            pending = [None]

            def flush_pending():
                if pending[0] is not None:
                    p2x, jx, cex = pending[0]
                    emit_normalize(p2x, jx, cex)
                    if p2x == NP - 1:
                        emit_out_chunks(jx)
                    pending[0] = None

            def emit_group(p2, j):
                h0, h1 = 2 * p2, 2 * p2 + 1
                n_kk = min(KK, 4 * j + 4)
                ce = [
                    psC.tile([D + 1, NQ], F32, tag=f"ce{hh}{j % 2}",
                             name=f"ce{hh}_{p2}_{j}")
                    for hh in range(2)
                ]
                for i in range(n_kk):
                    t = i - 4 * j  # >= 0 -> diagonal (partial) tile
                    q0 = P * t if t > 0 else 0
                    nq = NQ - q0
                    for hh, hloc in enumerate((h0, h1)):
                        sps = psS.tile([P, NQ], F32, tag="sc2",
                                       name=f"s2_{p2}_{j}_{i}_{hh}")
                        nc.tensor.matmul(
                            sps[:, 0:nq],
                            kT[ds(hh * D, D), p2, ts(i, P)],
                            qT[ds(hh * D, D), p2, ds(j * NQ + q0, nq)],
                            start=True, stop=True,
                        )
                        at = apool.tile([P, NQ], BF16, tag="at")
                        nc.scalar.activation(at[:, 0:nq], sps[:, 0:nq], AF.Exp)
                        if t >= 0:
                            nc.gpsimd.affine_select(
                                out=at[:, 0:nq], in_=at[:, 0:nq],
                                compare_op=mybir.AluOpType.is_ge,
                                fill=0.0, base=P * t - q0,
                                pattern=[[1, nq]], channel_multiplier=-1,
                            )
                        nc.tensor.matmul(
                            ce[hh][0:D + 1, ds(q0, nq)],
                            v_pad[:, i, hloc, :],
                            at[:, 0:nq],
                            start=(i == 0), stop=(i == n_kk - 1),
                        )
                    if i == min(3, n_kk - 1):
                        flush_pending()
                flush_pending()
                pending[0] = (p2, j, ce)

            VDELAY = 4
            for sc in range(S // P):
                if sc in xf_pre:
                    xf = xf_pre.pop(sc)
                else:
                    xf = stg.tile([P, E], BF16, tag="xstg", name=f"xf_{sc}")
                    nc.sync.dma_start(xf[:], x_b[ts(sc, P), :])
                if sc == 2:
                    nc.sync.dma_start(woT[:], wo_in[:])
                for e4 in range(EC // 4):
                    pt = psS.tile([P, 4, P], BF16, tag="sc2", name=f"ptx_{sc}_{e4}")
                    for k in range(4):
                        nc.tensor.transpose(
                            pt[:, k, :], xf[:, ts(4 * e4 + k, P)], identb[:]
                        )
                    nc.vector.tensor_copy(xT[:, 4 * e4:4 * e4 + 4, ts(sc, P)], pt[:])
                if sc >= VDELAY:
                    emit_v(sc - VDELAY)
                if sc % 4 == 3 and sc // 4 > 0:
                    emit_qk(0, sc // 4 - 1)
                if sc == 9:
                    emit_group(0, 0)
                if sc == 13:
                    emit_group(0, 1)
            for sc in range(S // P - VDELAY, S // P):
                emit_v(sc)
            emit_qk(0, J - 1)

            # ---- remaining attention groups (pair 0 groups 0-1 were emitted
            # inside the x loop); next pair's q/k interleaved ----
            for p2 in range(NP):
                for j in range(J):
                    if p2 == 0 and j < 2:
                        emit_qk(1, j)
                        continue
                    emit_group(p2, j)
                    if p2 + 1 < NP:
                        emit_qk(p2 + 1, j)
            flush_pending()

    nc.finalize()
    return nc


_NC_CACHE = {}


def _get_nc():
    key = "mha"
    if key not in _NC_CACHE:
        _NC_CACHE[key] = build_mha_nc()
    return _NC_CACHE[key]


def _arr_wqk(w, bf16):
    # [H, E, D] -> [P, NP, EC, 2*D] pair-packed lhsT layout
    H, E, D = w.shape
    P = 128
    v = w.astype(bf16).reshape(H // 2, 2, E // P, P, D)
    v = v.transpose(3, 0, 2, 1, 4)  # [P, NP, EC, 2, D]
    return np.ascontiguousarray(v.reshape(P, H // 2, E // P, 2 * D))


def _arr_wv(w, bf16):
    # [H, E, D] -> [P, EC, H, D]
    H, E, D = w.shape
    P = 128
    v = w.astype(bf16).reshape(H, E // P, P, D)
    return np.ascontiguousarray(v.transpose(2, 1, 0, 3))


def _arr_wo(w, bf16):
    # [E, HD] -> [P, HC, E] with woT[p, hc, e] = Wo[e, hc*128+p]
    E, HD = w.shape
    P = 128
    v = w.astype(bf16).T.reshape(HD // P, P, E)
    return np.ascontiguousarray(v.transpose(1, 0, 2))


def kernel(x, Wq, Wk, Wv, Wo, bo, _runner_kwargs=None):
    import ml_dtypes
    bf16 = ml_dtypes.bfloat16
    x = np.asarray(x, dtype=np.float32)
    Wq = np.asarray(Wq, dtype=np.float32)
    Wk = np.asarray(Wk, dtype=np.float32)
    Wv = np.asarray(Wv, dtype=np.float32)
    Wo = np.asarray(Wo, dtype=np.float32)
    bo = np.asarray(bo, dtype=np.float32)

    HPC = NUM_HEADS // 2  # heads per core
    HDS = HPC * HEAD      # concat-dim slice per core

    nc = _get_nc()
    xbs = [np.ascontiguousarray(x[b].astype(bf16)) for b in range(BATCH)]
    in_maps = []
    for c in range(N_CORES):
        b, g = c // 2, c % 2
        hs = slice(g * HPC, (g + 1) * HPC)
        in_maps.append({
            "x_b": xbs[b],
            "wq_in": _arr_wqk(Wq[hs], bf16),
            "wk_in": _arr_wqk(Wk[hs], bf16),
            "wv_in": _arr_wv(Wv[hs], bf16),
            "wo_in": _arr_wo(Wo[:, g * HDS:(g + 1) * HDS], bf16),
        })

    kw = dict(_runner_kwargs or {})
    res = bass_utils.run_bass_kernel_spmd(
        nc, in_maps, core_ids=list(range(N_CORES)), **kw
    )

    out = np.empty((BATCH, SEQ, EMB), dtype=np.float32)
    for b in range(BATCH):
        out[b] = res.results[2 * b]["out_p"] + res.results[2 * b + 1]["out_p"] + bo
    if kw.get("trace"):
        kernel.last_results = res
    return out


# revision 31
# speedup vs baseline: 1.1618x; 1.0230x over previous
"""Self-contained TRN2 Bass kernel for the 16-head MHA problem.

kernel(**inputs) takes FULL inputs (x [4,2048,1024], Wq/Wk/Wv [16,1024,64],
Wo [1024,1024], bo [1024]) and returns the FULL output [4,2048,1024] f32.

Sharding over 8 NeuronCores: core c handles batch b = c//2 and head group
g = c%2 (8 of 16 heads) — tensor parallel over heads with the output
projection's input dim sharded; the 2-way partial-sum reduce per batch and
the bias add happen host-side on the gathered results.
"""
import sys

for _p in ("/opt/trn_rl_repo",):
    if _p not in sys.path:
        sys.path.insert(0, _p)

import numpy as np
import concourse.bass as bass
import concourse.mybir as mybir
from concourse import bacc
from concourse.bass import ts, ds
from concourse.masks import make_identity
from concourse.tile import TileContext
from concourse.vector_clock import ScopedClock
from concourse import bass_utils

F32 = mybir.dt.float32
BF16 = mybir.dt.bfloat16
AF = mybir.ActivationFunctionType

NUM_HEADS = 16
EMB = 1024
HEAD = 64
SEQ = 2048
BATCH = 4
N_CORES = 8


class TC(TileContext):
    """TileContext whose final drain splits its sem waits across SP NOPs —
    the CTRL instruction encoding holds only one wait and this env's Tile
    puts the whole global clock on the tail drain."""

    def _drain_and_barrier(self, tick_clock, wait_clock):
        nc = self.nc
        dummy = nc.sync.nop(nofuse=True)
        wait_clock.add_sem_waits(dummy.ins, ScopedClock({None: tick_clock.global_clock}))
        si = dummy.ins.sync_info
        waits = list(si.on_wait) if si is not None else []
        if len(waits) > 1:
            si.on_wait = waits[:1]
            sem_by_name = {h.name: h for h in self.sems.allocated().values()}
            for w in waits[1:]:
                nop = nc.sync.nop(nofuse=True)
                nop._wait_ge(sem_by_name[w.ant_name], w.wait_value)
        nc.sync.drain()
        nc.all_engine_barrier()
        popped = nc._tile_sem_poison_stack.pop()
        assert popped is self._sem_poison
        nc.clear_and_free_semaphores(list(self.sems.allocated().values()))
        nc.all_engine_barrier()


def build_mha_nc(S=SEQ, E=EMB, D=HEAD, H=NUM_HEADS // 2):
    """Single-core SPMD program; H = heads per core (pair-packed).

    Fully transposed formulation:
      xT (PE-transposed once), qT/kT per pair (q pre-scaled by 1/sqrt(D)),
      scoresT = kT-slice.T @ qT-slice (K=64 row groups, both heads of a
      pair into one 2-bank PSUM tile),
      attnT = exp(scoresT) (scores bounded, no max subtraction; one ACT op
      and one causal affine_select per pair-tile),
      ctxT+denominator = [v|1].T @ attnT accumulated over k chunks,
      reciprocal of the denominator via DMA-reshape to [128, x] so all DVE
      lanes work, broadcast along d via a K=1 ones matmul,
      out = concatT-chunk.T @ WoT-chunk accumulated over head-dim chunks.

    Scheduling structure: q/k projections for pair p+1 are interleaved
    into pair p's attention groups to fill PE gaps; ce accumulators are
    tagged by q-chunk parity so a group's normalization tail overlaps the
    next group's matmuls.
    """
    P = 128
    EC = E // P
    NQ = 512
    J = S // NQ
    KK = S // P
    NP = H // 2
    HD = H * D
    HC = HD // P
    NE = min(512, E)
    JE = E // NE
    RQ = NQ // 64          # reshape width for the two-row reciprocal trick

    nc = bacc.Bacc("TRN2", target_bir_lowering=False, debug=False)
    # host pre-casts x to bf16 and pre-arranges the (small) weights into
    # their on-chip layouts, halving the startup DMA bytes
    x_b = nc.dram_tensor("x_b", [S, E], BF16, kind="ExternalInput")
    wq_in = nc.dram_tensor("wq_in", [P, NP, EC, 2 * D], BF16, kind="ExternalInput")
    wk_in = nc.dram_tensor("wk_in", [P, NP, EC, 2 * D], BF16, kind="ExternalInput")
    wv_in = nc.dram_tensor("wv_in", [P, EC, H, D], BF16, kind="ExternalInput")
    wo_in = nc.dram_tensor("wo_in", [P, HC, E], BF16, kind="ExternalInput")
    out_p = nc.dram_tensor("out_p", [S, E], F32, kind="ExternalOutput")

    scale = D ** -0.5

    with TC(nc) as tc:
        with (
            tc.tile_pool(name="const", bufs=1) as cpool,
            tc.tile_pool(name="persist", bufs=1) as pers,
            tc.tile_pool(name="stage", bufs=5) as stg,
            tc.tile_pool(name="attn", bufs=6) as apool,
            tc.tile_pool(name="small", bufs=3) as spool,
            tc.tile_pool(name="psS", bufs=4, space="PSUM") as psS,
            tc.tile_pool(name="psC", bufs=1, space="PSUM") as psC,
        ):
            identb = cpool.tile([P, P], BF16, tag="identb")
            make_identity(nc, identb[:])
            ones_col = cpool.tile([1, D], mybir.dt.float16, tag="ones")
            nc.gpsimd.memset(ones_col[:], 1.0)

            xT = pers.tile([P, EC, S], BF16, tag="xT")
            qT = pers.tile([P, NP, S], BF16, tag="qT")
            kT = pers.tile([P, NP, S], BF16, tag="kT")
            v_pad = pers.tile([P, KK, H, D + 1], BF16, tag="vp")
            woT = pers.tile([P, HC, E], BF16, tag="woT")
            concatT = pers.tile([P, NP, S], BF16, tag="concT")
            wq_bf = pers.tile([P, NP, EC, 2 * D], BF16, tag="wq")
            wk_bf = pers.tile([P, NP, EC, 2 * D], BF16, tag="wk")
            wv_bf = pers.tile([P, EC, H, D], BF16, tag="wv")

            nc.gpsimd.memset(v_pad[:, :, :, D:D + 1], 1.0)

            # ---- Phase A: stream bf16 x with transposes, v and pair-0 q/k
            # fused in; weights land via single pre-arranged DMAs ----
            xf_pre = {}
            for sc in range(4):
                xf = stg.tile([P, E], BF16, tag="xstg", name=f"xf_pre{sc}")
                nc.sync.dma_start(xf[:], x_b[ts(sc, P), :])
                xf_pre[sc] = xf
            nc.sync.dma_start(wv_bf[:], wv_in[:])
            nc.sync.dma_start(wq_bf[:], wq_in[:])
            nc.sync.dma_start(wk_bf[:], wk_in[:])

            def emit_qk(p2, sc):
                # sc indexes NQ-wide chunks (two per call site index)
                for w_sb, dst, sc_mul in ((wq_bf, qT, scale), (wk_bf, kT, 1.0)):
                    acc = psS.tile(
                        [P, NQ], F32, tag="sc2",
                        name=f"qk_{p2}_{sc}_{0 if dst is qT else 1}",
                    )
                    for ec in range(EC):
                        nc.tensor.matmul(
                            acc[:],
                            w_sb[:, p2, ec, :],
                            xT[:, ec, ts(sc, NQ)],
                            start=(ec == 0), stop=(ec == EC - 1),
                        )
                    if sc_mul != 1.0:
                        nc.vector.tensor_scalar_mul(
                            dst[:, p2, ts(sc, NQ)], acc[:], sc_mul
                        )
                    else:
                        nc.vector.tensor_copy(dst[:, p2, ts(sc, NQ)], acc[:])

            def emit_v(sc):
                acc = psS.tile([P, HD], F32, tag="sc2", name=f"vacc_{sc}")
                for ec in range(EC):
                    nc.tensor.matmul(
                        acc[:],
                        xT[:, ec, ts(sc, P)],
                        wv_bf[:, ec, :, :].rearrange("p h d -> p (h d)"),
                        start=(ec == 0), stop=(ec == EC - 1),
                    )
                nc.vector.tensor_copy(
                    v_pad[:, sc, :, 0:D],
                    acc[:].rearrange("p (h d) -> p h d", d=D),
                )

            def emit_out_chunks(j):
                for sc in range(4 * j, 4 * j + 4):
                    for n in range(JE):
                        acc = psS.tile([P, NE], F32, tag="sc2",
                                       name=f"oacc_{sc}_{n}")
                        for hc in range(HC):
                            nc.tensor.matmul(
                                acc[:],
                                concatT[:, hc, ts(sc, P)],
                                woT[:, hc, ts(n, NE)],
                                start=(hc == 0), stop=(hc == HC - 1),
                            )
                        ot = stg.tile([P, NE], F32, tag="ostg")
                        nc.vector.tensor_copy(ot[:], acc[:])
                        nc.sync.dma_start(out_p[ts(sc, P), ts(n, NE)], ot[:])

            def emit_normalize(p2, j, ce):
                dens = [
                    spool.tile([1, NQ], F32, tag=f"den{hh}",
                               name=f"den{hh}_{p2}_{j}")
                    for hh in range(2)
                ]
                for hh in range(2):
                    nc.vector.tensor_copy(dens[hh][:], ce[hh][ds(D, 1), :])
                den_rs = spool.tile([P, RQ], F32, tag="denrs")
                for hh in range(2):
                    nc.sync.dma_start(
                        den_rs[ds(hh * (P // 2), P // 2), :], dens[hh][:]
                    )
                rc = spool.tile([P, RQ], F32, tag="rc")
                nc.vector.reciprocal(rc[:], den_rs[:])
                rch = spool.tile([P, RQ], mybir.dt.float16, tag="rch")
                nc.vector.tensor_copy(rch[:], rc[:])
                recips = [
                    spool.tile([1, NQ], mybir.dt.float16, tag=f"recip{hh}",
                               name=f"recip{hh}_{p2}_{j}")
                    for hh in range(2)
                ]
                for hh in range(2):
                    nc.sync.dma_start(
                        recips[hh][:], rch[ds(hh * (P // 2), P // 2), :]
                    )
                for hh in range(2):
                    bc = psS.tile([D, NQ], F32, tag="sc2", name=f"bc_{p2}_{j}_{hh}")
                    nc.tensor.matmul(
                        bc[:], ones_col[:], recips[hh][:],
                        start=True, stop=True,
                    )
                    bc_sb = spool.tile([D, NQ], F32, tag="bcsb")
                    nc.vector.tensor_copy(bc_sb[:], bc[:])
                    nc.vector.tensor_mul(
                        concatT[ds(hh * D, D), p2, ts(j, NQ)],
                        ce[hh][0:D, :], bc_sb[:],
                    )

            pending = [None]

            def flush_pending():
                if pending[0] is not None:
                    p2x, jx, cex = pending[0]
                    emit_normalize(p2x, jx, cex)
                    if p2x == NP - 1:
                        emit_out_chunks(jx)
                    pending[0] = None

            def emit_group(p2, j):
                h0, h1 = 2 * p2, 2 * p2 + 1
                n_kk = min(KK, 4 * j + 4)
                ce = [
                    psC.tile([D + 1, NQ], F32, tag=f"ce{hh}{j % 2}",
                             name=f"ce{hh}_{p2}_{j}")
                    for hh in range(2)
                ]
                for i in range(n_kk):
                    t = i - 4 * j  # >= 0 -> diagonal (partial) tile
                    q0 = P * t if t > 0 else 0
                    nq = NQ - q0
                    for hh, hloc in enumerate((h0, h1)):
                        sps = psS.tile([P, NQ], F32, tag="sc2",
                                       name=f"s2_{p2}_{j}_{i}_{hh}")
                        nc.tensor.matmul(
                            sps[:, 0:nq],
                            kT[ds(hh * D, D), p2, ts(i, P)],
                            qT[ds(hh * D, D), p2, ds(j * NQ + q0, nq)],
                            start=True, stop=True,
                        )
                        at = apool.tile([P, NQ], BF16, tag="at")
                        nc.scalar.activation(at[:, 0:nq], sps[:, 0:nq], AF.Exp)
                        if t >= 0:
                            nc.gpsimd.affine_select(
                                out=at[:, 0:nq], in_=at[:, 0:nq],
                                compare_op=mybir.AluOpType.is_ge,
                                fill=0.0, base=P * t - q0,
                                pattern=[[1, nq]], channel_multiplier=-1,
                            )
                        nc.tensor.matmul(
                            ce[hh][0:D + 1, ds(q0, nq)],
                            v_pad[:, i, hloc, :],
                            at[:, 0:nq],
                            start=(i == 0), stop=(i == n_kk - 1),
                        )
                    if i == min(3, n_kk - 1):
                        flush_pending()
                flush_pending()
                pending[0] = (p2, j, ce)

            VDELAY = 4
            for sc in range(S // P):
                if sc in xf_pre:
                    xf = xf_pre.pop(sc)
                else:
                    xf = stg.tile([P, E], BF16, tag="xstg", name=f"xf_{sc}")
                    nc.sync.dma_start(xf[:], x_b[ts(sc, P), :])
                if sc == 2:
                    nc.sync.dma_start(woT[:], wo_in[:])
                for e4 in range(EC // 4):
                    pt = psS.tile([P, 4, P], BF16, tag="sc2", name=f"ptx_{sc}_{e4}")
                    for k in range(4):
                        nc.tensor.transpose(
                            pt[:, k, :], xf[:, ts(4 * e4 + k, P)], identb[:]
                        )
                    nc.vector.tensor_copy(xT[:, 4 * e4:4 * e4 + 4, ts(sc, P)], pt[:])
                if sc >= VDELAY:
                    emit_v(sc - VDELAY)
                if sc % 4 == 3 and sc // 4 > 0:
                    emit_qk(0, sc // 4 - 1)
                if sc == 9:
                    emit_group(0, 0)
                if sc == 13:
                    emit_group(0, 1)
            for sc in range(S // P - VDELAY, S // P):
                emit_v(sc)
            emit_qk(0, J - 1)

            # ---- remaining attention groups (pair 0 groups 0-1 were emitted
            # inside the x loop); next pair's q/k interleaved ----
            for p2 in range(NP):
                for j in range(J):
                    if p2 == 0 and j < 2:
                        emit_qk(1, j)
                        continue
                    emit_group(p2, j)
                    if p2 + 1 < NP:
                        emit_qk(p2 + 1, j)
            flush_pending()

    nc.finalize()
    return nc


_NC_CACHE = {}


def _get_nc():
    key = "mha"
    if key not in _NC_CACHE:
        _NC_CACHE[key] = build_mha_nc()
    return _NC_CACHE[key]


def _arr_wqk(w, bf16):
    # [H, E, D] -> [P, NP, EC, 2*D] pair-packed lhsT layout
    H, E, D = w.shape
    P = 128
    v = w.astype(bf16).reshape(H // 2, 2, E // P, P, D)
    v = v.transpose(3, 0, 2, 1, 4)  # [P, NP, EC, 2, D]
    return np.ascontiguousarray(v.reshape(P, H // 2, E // P, 2 * D))


def _arr_wv(w, bf16):
    # [H, E, D] -> [P, EC, H, D]
    H, E, D = w.shape
    P = 128
    v = w.astype(bf16).reshape(H, E // P, P, D)
    return np.ascontiguousarray(v.transpose(2, 1, 0, 3))


def _arr_wo(w, bf16):
    # [E, HD] -> [P, HC, E] with woT[p, hc, e] = Wo[e, hc*128+p]
    E, HD = w.shape
    P = 128
    v = w.astype(bf16).T.reshape(HD // P, P, E)
    return np.ascontiguousarray(v.transpose(1, 0, 2))


def kernel(x, Wq, Wk, Wv, Wo, bo, _runner_kwargs=None):
    import ml_dtypes
    bf16 = ml_dtypes.bfloat16
    x = np.asarray(x, dtype=np.float32)
    Wq = np.asarray(Wq, dtype=np.float32)
    Wk = np.asarray(Wk, dtype=np.float32)
    Wv = np.asarray(Wv, dtype=np.float32)
    Wo = np.asarray(Wo, dtype=np.float32)
    bo = np.asarray(bo, dtype=np.float32)

    HPC = NUM_HEADS // 2  # heads per core
    HDS = HPC * HEAD      # concat-dim slice per core

    nc = _get_nc()
    xbs = [np.ascontiguousarray(x[b].astype(bf16)) for b in range(BATCH)]
    in_maps = []
    for c in range(N_CORES):
        b, g = c // 2, c % 2
        hs = slice(g * HPC, (g + 1) * HPC)
        in_maps.append({
            "x_b": xbs[b],
            "wq_in": _arr_wqk(Wq[hs], bf16),
            "wk_in": _arr_wqk(Wk[hs], bf16),
            "wv_in": _arr_wv(Wv[hs], bf16),
            "wo_in": _arr_wo(Wo[:, g * HDS:(g + 1) * HDS], bf16),
        })

    kw = dict(_runner_kwargs or {})
    res = bass_utils.run_bass_kernel_spmd(
        nc, in_maps, core_ids=list(range(N_CORES)), **kw
    )

    out = np.empty((BATCH, SEQ, EMB), dtype=np.float32)
    for b in range(BATCH):
        out[b] = res.results[2 * b]["out_p"] + res.results[2 * b + 1]["out_p"] + bo
    if kw.get("trace"):
        kernel.last_results = res
    return out


# revision 32
# speedup vs baseline: 1.2310x; 1.0595x over previous
"""Self-contained TRN2 Bass kernel for the 16-head MHA problem.

kernel(**inputs) takes FULL inputs (x [4,2048,1024], Wq/Wk/Wv [16,1024,64],
Wo [1024,1024], bo [1024]) and returns the FULL output [4,2048,1024] f32.

Sharding over 8 NeuronCores: core c handles batch b = c//2 and head group
g = c%2 (8 of 16 heads) — tensor parallel over heads with the output
projection's input dim sharded; the 2-way partial-sum reduce per batch and
the bias add happen host-side on the gathered results.
"""
import sys

for _p in ("/opt/trn_rl_repo",):
    if _p not in sys.path:
        sys.path.insert(0, _p)

import numpy as np
import concourse.bass as bass
import concourse.mybir as mybir
from concourse import bacc
from concourse.bass import ts, ds
from concourse.tile import TileContext
from concourse.vector_clock import ScopedClock
from concourse import bass_utils

F32 = mybir.dt.float32
BF16 = mybir.dt.bfloat16
AF = mybir.ActivationFunctionType

NUM_HEADS = 16
EMB = 1024
HEAD = 64
SEQ = 2048
BATCH = 4
N_CORES = 8


class TC(TileContext):
    """TileContext whose final drain splits its sem waits across SP NOPs —
    the CTRL instruction encoding holds only one wait and this env's Tile
    puts the whole global clock on the tail drain."""

    def _drain_and_barrier(self, tick_clock, wait_clock):
        nc = self.nc
        dummy = nc.sync.nop(nofuse=True)
        wait_clock.add_sem_waits(dummy.ins, ScopedClock({None: tick_clock.global_clock}))
        si = dummy.ins.sync_info
        waits = list(si.on_wait) if si is not None else []
        if len(waits) > 1:
            si.on_wait = waits[:1]
            sem_by_name = {h.name: h for h in self.sems.allocated().values()}
            for w in waits[1:]:
                nop = nc.sync.nop(nofuse=True)
                nop._wait_ge(sem_by_name[w.ant_name], w.wait_value)
        nc.sync.drain()
        nc.all_engine_barrier()
        popped = nc._tile_sem_poison_stack.pop()
        assert popped is self._sem_poison
        nc.clear_and_free_semaphores(list(self.sems.allocated().values()))
        nc.all_engine_barrier()


def build_mha_nc(S=SEQ, E=EMB, D=HEAD, H=NUM_HEADS // 2):
    """Single-core SPMD program; H = heads per core (pair-packed).

    Fully transposed formulation:
      xT (PE-transposed once), qT/kT per pair (q pre-scaled by 1/sqrt(D)),
      scoresT = kT-slice.T @ qT-slice (K=64 row groups, both heads of a
      pair into one 2-bank PSUM tile),
      attnT = exp(scoresT) (scores bounded, no max subtraction; one ACT op
      and one causal affine_select per pair-tile),
      ctxT+denominator = [v|1].T @ attnT accumulated over k chunks,
      reciprocal of the denominator via DMA-reshape to [128, x] so all DVE
      lanes work, broadcast along d via a K=1 ones matmul,
      out = concatT-chunk.T @ WoT-chunk accumulated over head-dim chunks.

    Scheduling structure: q/k projections for pair p+1 are interleaved
    into pair p's attention groups to fill PE gaps; ce accumulators are
    tagged by q-chunk parity so a group's normalization tail overlaps the
    next group's matmuls.
    """
    P = 128
    EC = E // P
    NQ = 512
    J = S // NQ
    KK = S // P
    NP = H // 2
    HD = H * D
    HC = HD // P
    NE = min(512, E)
    JE = E // NE
    RQ = NQ // 64          # reshape width for the two-row reciprocal trick

    nc = bacc.Bacc("TRN2", target_bir_lowering=False, debug=False)
    # host pre-casts x to bf16 and pre-arranges the (small) weights into
    # their on-chip layouts, halving the startup DMA bytes
    xT_in = nc.dram_tensor("xT_in", [P, EC, S], BF16, kind="ExternalInput")
    wq_in = nc.dram_tensor("wq_in", [P, NP, EC, 2 * D], BF16, kind="ExternalInput")
    wk_in = nc.dram_tensor("wk_in", [P, NP, EC, 2 * D], BF16, kind="ExternalInput")
    wv_in = nc.dram_tensor("wv_in", [P, EC, H, D], BF16, kind="ExternalInput")
    wo_in = nc.dram_tensor("wo_in", [P, HC, E], BF16, kind="ExternalInput")
    out_p = nc.dram_tensor("out_p", [S, E], F32, kind="ExternalOutput")

    scale = D ** -0.5

    with TC(nc) as tc:
        with (
            tc.tile_pool(name="const", bufs=1) as cpool,
            tc.tile_pool(name="persist", bufs=1) as pers,
            tc.tile_pool(name="stage", bufs=5) as stg,
            tc.tile_pool(name="attn", bufs=6) as apool,
            tc.tile_pool(name="small", bufs=3) as spool,
            tc.tile_pool(name="psS", bufs=4, space="PSUM") as psS,
            tc.tile_pool(name="psC", bufs=1, space="PSUM") as psC,
        ):
            ones_col = cpool.tile([1, D], mybir.dt.float16, tag="ones")
            nc.gpsimd.memset(ones_col[:], 1.0)

            xT = pers.tile([P, EC, S], BF16, tag="xT")
            qT = pers.tile([P, NP, S], BF16, tag="qT")
            kT = pers.tile([P, NP, S], BF16, tag="kT")
            v_pad = pers.tile([P, KK, H, D + 1], BF16, tag="vp")
            woT = pers.tile([P, HC, E], BF16, tag="woT")
            concatT = pers.tile([P, NP, S], BF16, tag="concT")
            wq_bf = pers.tile([P, NP, EC, 2 * D], BF16, tag="wq")
            wk_bf = pers.tile([P, NP, EC, 2 * D], BF16, tag="wk")
            wv_bf = pers.tile([P, EC, H, D], BF16, tag="wv")

            nc.gpsimd.memset(v_pad[:, :, :, D:D + 1], 1.0)

            # ---- Phase A: stream host-transposed bf16 xT in fine slices,
            # with v and pair-0 q/k projections fused in; weights land via
            # single pre-arranged DMAs queued behind the first slices ----
            for sc in range(4):
                nc.sync.dma_start(xT[:, :, ts(sc, P)], xT_in[:, :, ts(sc, P)])
            nc.sync.dma_start(wv_bf[:], wv_in[:])
            nc.sync.dma_start(wq_bf[:], wq_in[:])
            nc.sync.dma_start(wk_bf[:], wk_in[:])

            def emit_qk(p2, sc):
                # sc indexes NQ-wide chunks (two per call site index)
                for w_sb, dst, sc_mul in ((wq_bf, qT, scale), (wk_bf, kT, 1.0)):
                    acc = psS.tile(
                        [P, NQ], F32, tag="sc2",
                        name=f"qk_{p2}_{sc}_{0 if dst is qT else 1}",
                    )
                    for ec in range(EC):
                        nc.tensor.matmul(
                            acc[:],
                            w_sb[:, p2, ec, :],
                            xT[:, ec, ts(sc, NQ)],
                            start=(ec == 0), stop=(ec == EC - 1),
                        )
                    if sc_mul != 1.0:
                        nc.vector.tensor_scalar_mul(
                            dst[:, p2, ts(sc, NQ)], acc[:], sc_mul
                        )
                    else:
                        nc.vector.tensor_copy(dst[:, p2, ts(sc, NQ)], acc[:])

            def emit_v(sc):
                acc = psS.tile([P, HD], F32, tag="sc2", name=f"vacc_{sc}")
                for ec in range(EC):
                    nc.tensor.matmul(
                        acc[:],
                        xT[:, ec, ts(sc, P)],
                        wv_bf[:, ec, :, :].rearrange("p h d -> p (h d)"),
                        start=(ec == 0), stop=(ec == EC - 1),
                    )
                nc.vector.tensor_copy(
                    v_pad[:, sc, :, 0:D],
                    acc[:].rearrange("p (h d) -> p h d", d=D),
                )

            def emit_out_chunks(j):
                for sc in range(4 * j, 4 * j + 4):
                    for n in range(JE):
                        acc = psS.tile([P, NE], F32, tag="sc2",
                                       name=f"oacc_{sc}_{n}")
                        for hc in range(HC):
                            nc.tensor.matmul(
                                acc[:],
                                concatT[:, hc, ts(sc, P)],
                                woT[:, hc, ts(n, NE)],
                                start=(hc == 0), stop=(hc == HC - 1),
                            )
                        ot = stg.tile([P, NE], F32, tag="ostg")
                        nc.vector.tensor_copy(ot[:], acc[:])
                        nc.sync.dma_start(out_p[ts(sc, P), ts(n, NE)], ot[:])

            def emit_normalize(p2, j, ce):
                dens = [
                    spool.tile([1, NQ], F32, tag=f"den{hh}",
                               name=f"den{hh}_{p2}_{j}")
                    for hh in range(2)
                ]
                for hh in range(2):
                    nc.vector.tensor_copy(dens[hh][:], ce[hh][ds(D, 1), :])
                den_rs = spool.tile([P, RQ], F32, tag="denrs")
                for hh in range(2):
                    nc.sync.dma_start(
                        den_rs[ds(hh * (P // 2), P // 2), :], dens[hh][:]
                    )
                rc = spool.tile([P, RQ], F32, tag="rc")
                nc.vector.reciprocal(rc[:], den_rs[:])
                rch = spool.tile([P, RQ], mybir.dt.float16, tag="rch")
                nc.vector.tensor_copy(rch[:], rc[:])
                recips = [
                    spool.tile([1, NQ], mybir.dt.float16, tag=f"recip{hh}",
                               name=f"recip{hh}_{p2}_{j}")
                    for hh in range(2)
                ]
                for hh in range(2):
                    nc.sync.dma_start(
                        recips[hh][:], rch[ds(hh * (P // 2), P // 2), :]
                    )
                for hh in range(2):
                    bc = psS.tile([D, NQ], F32, tag="sc2", name=f"bc_{p2}_{j}_{hh}")
                    nc.tensor.matmul(
                        bc[:], ones_col[:], recips[hh][:],
                        start=True, stop=True,
                    )
                    bc_sb = spool.tile([D, NQ], F32, tag="bcsb")
                    nc.vector.tensor_copy(bc_sb[:], bc[:])
                    nc.vector.tensor_mul(
                        concatT[ds(hh * D, D), p2, ts(j, NQ)],
                        ce[hh][0:D, :], bc_sb[:],
                    )

            pending = [None]

            def flush_pending():
                if pending[0] is not None:
                    p2x, jx, cex = pending[0]
                    emit_normalize(p2x, jx, cex)
                    if p2x == NP - 1:
                        emit_out_chunks(jx)
                    pending[0] = None

            def emit_group(p2, j):
                h0, h1 = 2 * p2, 2 * p2 + 1
                n_kk = min(KK, 4 * j + 4)
                ce = [
                    psC.tile([D + 1, NQ], F32, tag=f"ce{hh}{j % 2}",
                             name=f"ce{hh}_{p2}_{j}")
                    for hh in range(2)
                ]
                for i in range(n_kk):
                    t = i - 4 * j  # >= 0 -> diagonal (partial) tile
                    q0 = P * t if t > 0 else 0
                    nq = NQ - q0
                    for hh, hloc in enumerate((h0, h1)):
                        sps = psS.tile([P, NQ], F32, tag="sc2",
                                       name=f"s2_{p2}_{j}_{i}_{hh}")
                        nc.tensor.matmul(
                            sps[:, 0:nq],
                            kT[ds(hh * D, D), p2, ts(i, P)],
                            qT[ds(hh * D, D), p2, ds(j * NQ + q0, nq)],
                            start=True, stop=True,
                        )
                        at = apool.tile([P, NQ], BF16, tag="at")
                        nc.scalar.activation(at[:, 0:nq], sps[:, 0:nq], AF.Exp)
                        if t >= 0:
                            nc.gpsimd.affine_select(
                                out=at[:, 0:nq], in_=at[:, 0:nq],
                                compare_op=mybir.AluOpType.is_ge,
                                fill=0.0, base=P * t - q0,
                                pattern=[[1, nq]], channel_multiplier=-1,
                            )
                        nc.tensor.matmul(
                            ce[hh][0:D + 1, ds(q0, nq)],
                            v_pad[:, i, hloc, :],
                            at[:, 0:nq],
                            start=(i == 0), stop=(i == n_kk - 1),
                        )
                    if i == min(3, n_kk - 1):
                        flush_pending()
                flush_pending()
                pending[0] = (p2, j, ce)

            VDELAY = 4
            for sc in range(S // P):
                if sc >= 4:
                    nc.sync.dma_start(xT[:, :, ts(sc, P)], xT_in[:, :, ts(sc, P)])
                if sc == 2:
                    nc.sync.dma_start(woT[:], wo_in[:])
                if sc >= VDELAY:
                    emit_v(sc - VDELAY)
                if sc % 4 == 3 and sc // 4 > 0:
                    emit_qk(0, sc // 4 - 1)
                if sc == 9:
                    emit_group(0, 0)
                if sc == 13:
                    emit_group(0, 1)
            for sc in range(S // P - VDELAY, S // P):
                emit_v(sc)
            emit_qk(0, J - 1)

            # ---- remaining attention groups (pair 0 groups 0-1 were emitted
            # inside the x loop); next pair's q/k interleaved ----
            for p2 in range(NP):
                for j in range(J):
                    if p2 == 0 and j < 2:
                        emit_qk(1, j)
                        continue
                    emit_group(p2, j)
                    if p2 + 1 < NP:
                        emit_qk(p2 + 1, j)
            flush_pending()

    nc.finalize()
    return nc


_NC_CACHE = {}


def _get_nc():
    key = "mha"
    if key not in _NC_CACHE:
        _NC_CACHE[key] = build_mha_nc()
    return _NC_CACHE[key]


def _arr_xT(xb, bf16):
    # [S, E] f32 -> [P, EC, S] bf16 with xT[p, ec, s] = x[s, ec*128+p]
    P, S, E = 128, xb.shape[0], xb.shape[1]
    xt = xb.astype(bf16).T.reshape(E // P, P, S)
    return np.ascontiguousarray(xt.transpose(1, 0, 2))


def _arr_wqk(w, bf16):
    # [H, E, D] -> [P, NP, EC, 2*D] pair-packed lhsT layout
    H, E, D = w.shape
    P = 128
    v = w.astype(bf16).reshape(H // 2, 2, E // P, P, D)
    v = v.transpose(3, 0, 2, 1, 4)  # [P, NP, EC, 2, D]
    return np.ascontiguousarray(v.reshape(P, H // 2, E // P, 2 * D))


def _arr_wv(w, bf16):
    # [H, E, D] -> [P, EC, H, D]
    H, E, D = w.shape
    P = 128
    v = w.astype(bf16).reshape(H, E // P, P, D)
    return np.ascontiguousarray(v.transpose(2, 1, 0, 3))


def _arr_wo(w, bf16):
    # [E, HD] -> [P, HC, E] with woT[p, hc, e] = Wo[e, hc*128+p]
    E, HD = w.shape
    P = 128
    v = w.astype(bf16).T.reshape(HD // P, P, E)
    return np.ascontiguousarray(v.transpose(1, 0, 2))


def kernel(x, Wq, Wk, Wv, Wo, bo, _runner_kwargs=None):
    import ml_dtypes
    bf16 = ml_dtypes.bfloat16
    x = np.asarray(x, dtype=np.float32)
    Wq = np.asarray(Wq, dtype=np.float32)
    Wk = np.asarray(Wk, dtype=np.float32)
    Wv = np.asarray(Wv, dtype=np.float32)
    Wo = np.asarray(Wo, dtype=np.float32)
    bo = np.asarray(bo, dtype=np.float32)

    HPC = NUM_HEADS // 2  # heads per core
    HDS = HPC * HEAD      # concat-dim slice per core

    nc = _get_nc()
    xbs = [_arr_xT(x[b], bf16) for b in range(BATCH)]
    in_maps = []
    for c in range(N_CORES):
        b, g = c // 2, c % 2
        hs = slice(g * HPC, (g + 1) * HPC)
        in_maps.append({
            "xT_in": xbs[b],
            "wq_in": _arr_wqk(Wq[hs], bf16),
            "wk_in": _arr_wqk(Wk[hs], bf16),
            "wv_in": _arr_wv(Wv[hs], bf16),
            "wo_in": _arr_wo(Wo[:, g * HDS:(g + 1) * HDS], bf16),
        })

    kw = dict(_runner_kwargs or {})
    res = bass_utils.run_bass_kernel_spmd(
        nc, in_maps, core_ids=list(range(N_CORES)), **kw
    )

    out = np.empty((BATCH, SEQ, EMB), dtype=np.float32)
    for b in range(BATCH):
        out[b] = res.results[2 * b]["out_p"] + res.results[2 * b + 1]["out_p"] + bo
    if kw.get("trace"):
        kernel.last_results = res
    return out


# revision 33
# speedup vs baseline: 1.2350x; 1.0033x over previous
"""Self-contained TRN2 Bass kernel for the 16-head MHA problem.

kernel(**inputs) takes FULL inputs (x [4,2048,1024], Wq/Wk/Wv [16,1024,64],
Wo [1024,1024], bo [1024]) and returns the FULL output [4,2048,1024] f32.

Sharding over 8 NeuronCores: core c handles batch b = c//2 and head group
g = c%2 (8 of 16 heads) — tensor parallel over heads with the output
projection's input dim sharded; the 2-way partial-sum reduce per batch and
the bias add happen host-side on the gathered results.
"""
import sys

for _p in ("/opt/trn_rl_repo",):
    if _p not in sys.path:
        sys.path.insert(0, _p)

import numpy as np
import concourse.bass as bass
import concourse.mybir as mybir
from concourse import bacc
from concourse.bass import ts, ds
from concourse.tile import TileContext
from concourse.vector_clock import ScopedClock
from concourse import bass_utils

F32 = mybir.dt.float32
BF16 = mybir.dt.bfloat16
AF = mybir.ActivationFunctionType

NUM_HEADS = 16
EMB = 1024
HEAD = 64
SEQ = 2048
BATCH = 4
N_CORES = 8


class TC(TileContext):
    """TileContext whose final drain splits its sem waits across SP NOPs —
    the CTRL instruction encoding holds only one wait and this env's Tile
    puts the whole global clock on the tail drain."""

    def _drain_and_barrier(self, tick_clock, wait_clock):
        nc = self.nc
        dummy = nc.sync.nop(nofuse=True)
        wait_clock.add_sem_waits(dummy.ins, ScopedClock({None: tick_clock.global_clock}))
        si = dummy.ins.sync_info
        waits = list(si.on_wait) if si is not None else []
        if len(waits) > 1:
            si.on_wait = waits[:1]
            sem_by_name = {h.name: h for h in self.sems.allocated().values()}
            for w in waits[1:]:
                nop = nc.sync.nop(nofuse=True)
                nop._wait_ge(sem_by_name[w.ant_name], w.wait_value)
        nc.sync.drain()
        nc.all_engine_barrier()
        popped = nc._tile_sem_poison_stack.pop()
        assert popped is self._sem_poison
        nc.clear_and_free_semaphores(list(self.sems.allocated().values()))
        nc.all_engine_barrier()


def build_mha_nc(S=SEQ, E=EMB, D=HEAD, H=NUM_HEADS // 2):
    """Single-core SPMD program; H = heads per core (pair-packed).

    Fully transposed formulation:
      xT (PE-transposed once), qT/kT per pair (q pre-scaled by 1/sqrt(D)),
      scoresT = kT-slice.T @ qT-slice (K=64 row groups, both heads of a
      pair into one 2-bank PSUM tile),
      attnT = exp(scoresT) (scores bounded, no max subtraction; one ACT op
      and one causal affine_select per pair-tile),
      ctxT+denominator = [v|1].T @ attnT accumulated over k chunks,
      reciprocal of the denominator via DMA-reshape to [128, x] so all DVE
      lanes work, broadcast along d via a K=1 ones matmul,
      out = concatT-chunk.T @ WoT-chunk accumulated over head-dim chunks.

    Scheduling structure: q/k projections for pair p+1 are interleaved
    into pair p's attention groups to fill PE gaps; ce accumulators are
    tagged by q-chunk parity so a group's normalization tail overlaps the
    next group's matmuls.
    """
    P = 128
    EC = E // P
    NQ = 512
    J = S // NQ
    KK = S // P
    NP = H // 2
    HD = H * D
    HC = HD // P
    NE = min(512, E)
    JE = E // NE
    RQ = NQ // 64          # reshape width for the two-row reciprocal trick

    nc = bacc.Bacc("TRN2", target_bir_lowering=False, debug=False)
    # host pre-casts x to bf16 and pre-arranges the (small) weights into
    # their on-chip layouts, halving the startup DMA bytes
    xT_in = nc.dram_tensor("xT_in", [P, EC, S], BF16, kind="ExternalInput")
    wq_in = nc.dram_tensor("wq_in", [P, NP, EC, 2 * D], BF16, kind="ExternalInput")
    wk_in = nc.dram_tensor("wk_in", [P, NP, EC, 2 * D], BF16, kind="ExternalInput")
    wv_in = nc.dram_tensor("wv_in", [P, EC, H, D], BF16, kind="ExternalInput")
    wo_in = nc.dram_tensor("wo_in", [P, HC, E], BF16, kind="ExternalInput")
    out_p = nc.dram_tensor("out_p", [S, E], F32, kind="ExternalOutput")

    scale = D ** -0.5

    with TC(nc) as tc:
        with (
            tc.tile_pool(name="const", bufs=1) as cpool,
            tc.tile_pool(name="persist", bufs=1) as pers,
            tc.tile_pool(name="stage", bufs=5) as stg,
            tc.tile_pool(name="attn", bufs=6) as apool,
            tc.tile_pool(name="small", bufs=3) as spool,
            tc.tile_pool(name="psS", bufs=4, space="PSUM") as psS,
            tc.tile_pool(name="psC", bufs=1, space="PSUM") as psC,
        ):
            ones_col = cpool.tile([1, D], mybir.dt.float16, tag="ones")
            nc.gpsimd.memset(ones_col[:], 1.0)

            xT = pers.tile([P, EC, S], BF16, tag="xT")
            qT = pers.tile([P, NP, S], BF16, tag="qT")
            kT = pers.tile([P, NP, S], BF16, tag="kT")
            v_pad = pers.tile([P, KK, H, D + 1], BF16, tag="vp")
            woT = pers.tile([P, HC, E], BF16, tag="woT")
            concatT = pers.tile([P, NP, S], BF16, tag="concT")
            wq_bf = pers.tile([P, NP, EC, 2 * D], BF16, tag="wq")
            wk_bf = pers.tile([P, NP, EC, 2 * D], BF16, tag="wk")
            wv_bf = pers.tile([P, EC, H, D], BF16, tag="wv")

            nc.gpsimd.memset(v_pad[:, :, :, D:D + 1], 1.0)

            # ---- Phase A: stream host-transposed bf16 xT in fine slices,
            # with v and pair-0 q/k projections fused in; weights land via
            # single pre-arranged DMAs queued behind the first slices ----
            nc.sync.dma_start(xT[:, :, ts(0, P)], xT_in[:, :, ts(0, P)])
            nc.sync.dma_start(wv_bf[:], wv_in[:])
            for sc in range(1, 4):
                nc.sync.dma_start(xT[:, :, ts(sc, P)], xT_in[:, :, ts(sc, P)])
            nc.sync.dma_start(wq_bf[:], wq_in[:])
            nc.sync.dma_start(wk_bf[:], wk_in[:])

            def emit_qk(p2, sc):
                # sc indexes NQ-wide chunks (two per call site index)
                for w_sb, dst, sc_mul in ((wq_bf, qT, scale), (wk_bf, kT, 1.0)):
                    acc = psS.tile(
                        [P, NQ], F32, tag="sc2",
                        name=f"qk_{p2}_{sc}_{0 if dst is qT else 1}",
                    )
                    for ec in range(EC):
                        nc.tensor.matmul(
                            acc[:],
                            w_sb[:, p2, ec, :],
                            xT[:, ec, ts(sc, NQ)],
                            start=(ec == 0), stop=(ec == EC - 1),
                        )
                    if sc_mul != 1.0:
                        nc.vector.tensor_scalar_mul(
                            dst[:, p2, ts(sc, NQ)], acc[:], sc_mul
                        )
                    else:
                        nc.vector.tensor_copy(dst[:, p2, ts(sc, NQ)], acc[:])

            def emit_v(sc):
                acc = psS.tile([P, HD], F32, tag="sc2", name=f"vacc_{sc}")
                for ec in range(EC):
                    nc.tensor.matmul(
                        acc[:],
                        xT[:, ec, ts(sc, P)],
                        wv_bf[:, ec, :, :].rearrange("p h d -> p (h d)"),
                        start=(ec == 0), stop=(ec == EC - 1),
                    )
                nc.vector.tensor_copy(
                    v_pad[:, sc, :, 0:D],
                    acc[:].rearrange("p (h d) -> p h d", d=D),
                )

            def emit_out_chunks(j):
                for sc in range(4 * j, 4 * j + 4):
                    for n in range(JE):
                        acc = psS.tile([P, NE], F32, tag="sc2",
                                       name=f"oacc_{sc}_{n}")
                        for hc in range(HC):
                            nc.tensor.matmul(
                                acc[:],
                                concatT[:, hc, ts(sc, P)],
                                woT[:, hc, ts(n, NE)],
                                start=(hc == 0), stop=(hc == HC - 1),
                            )
                        ot = stg.tile([P, NE], F32, tag="ostg")
                        nc.vector.tensor_copy(ot[:], acc[:])
                        nc.sync.dma_start(out_p[ts(sc, P), ts(n, NE)], ot[:])

            def emit_normalize(p2, j, ce):
                dens = [
                    spool.tile([1, NQ], F32, tag=f"den{hh}",
                               name=f"den{hh}_{p2}_{j}")
                    for hh in range(2)
                ]
                for hh in range(2):
                    nc.vector.tensor_copy(dens[hh][:], ce[hh][ds(D, 1), :])
                den_rs = spool.tile([P, RQ], F32, tag="denrs")
                for hh in range(2):
                    nc.sync.dma_start(
                        den_rs[ds(hh * (P // 2), P // 2), :], dens[hh][:]
                    )
                rc = spool.tile([P, RQ], F32, tag="rc")
                nc.vector.reciprocal(rc[:], den_rs[:])
                rch = spool.tile([P, RQ], mybir.dt.float16, tag="rch")
                nc.vector.tensor_copy(rch[:], rc[:])
                recips = [
                    spool.tile([1, NQ], mybir.dt.float16, tag=f"recip{hh}",
                               name=f"recip{hh}_{p2}_{j}")
                    for hh in range(2)
                ]
                for hh in range(2):
                    nc.sync.dma_start(
                        recips[hh][:], rch[ds(hh * (P // 2), P // 2), :]
                    )
                for hh in range(2):
                    bc = psS.tile([D, NQ], F32, tag="sc2", name=f"bc_{p2}_{j}_{hh}")
                    nc.tensor.matmul(
                        bc[:], ones_col[:], recips[hh][:],
                        start=True, stop=True,
                    )
                    bc_sb = spool.tile([D, NQ], F32, tag="bcsb")
                    nc.vector.tensor_copy(bc_sb[:], bc[:])
                    nc.vector.tensor_mul(
                        concatT[ds(hh * D, D), p2, ts(j, NQ)],
                        ce[hh][0:D, :], bc_sb[:],
                    )

            pending = [None]

            def flush_pending():
                if pending[0] is not None:
                    p2x, jx, cex = pending[0]
                    emit_normalize(p2x, jx, cex)
                    if p2x == NP - 1:
                        emit_out_chunks(jx)
                    pending[0] = None

            def emit_group(p2, j):
                h0, h1 = 2 * p2, 2 * p2 + 1
                n_kk = min(KK, 4 * j + 4)
                ce = [
                    psC.tile([D + 1, NQ], F32, tag=f"ce{hh}{j % 2}",
                             name=f"ce{hh}_{p2}_{j}")
                    for hh in range(2)
                ]
                for i in range(n_kk):
                    t = i - 4 * j  # >= 0 -> diagonal (partial) tile
                    q0 = P * t if t > 0 else 0
                    nq = NQ - q0
                    for hh, hloc in enumerate((h0, h1)):
                        sps = psS.tile([P, NQ], F32, tag="sc2",
                                       name=f"s2_{p2}_{j}_{i}_{hh}")
                        nc.tensor.matmul(
                            sps[:, 0:nq],
                            kT[ds(hh * D, D), p2, ts(i, P)],
                            qT[ds(hh * D, D), p2, ds(j * NQ + q0, nq)],
                            start=True, stop=True,
                        )
                        at = apool.tile([P, NQ], BF16, tag="at")
                        nc.scalar.activation(at[:, 0:nq], sps[:, 0:nq], AF.Exp)
                        if t >= 0:
                            nc.gpsimd.affine_select(
                                out=at[:, 0:nq], in_=at[:, 0:nq],
                                compare_op=mybir.AluOpType.is_ge,
                                fill=0.0, base=P * t - q0,
                                pattern=[[1, nq]], channel_multiplier=-1,
                            )
                        nc.tensor.matmul(
                            ce[hh][0:D + 1, ds(q0, nq)],
                            v_pad[:, i, hloc, :],
                            at[:, 0:nq],
                            start=(i == 0), stop=(i == n_kk - 1),
                        )
                    if i == min(3, n_kk - 1):
                        flush_pending()
                flush_pending()
                pending[0] = (p2, j, ce)

            VDELAY = 2
            for sc in range(S // P):
                if sc >= 4:
                    nc.sync.dma_start(xT[:, :, ts(sc, P)], xT_in[:, :, ts(sc, P)])
                if sc == 2:
                    nc.sync.dma_start(woT[:], wo_in[:])
                if sc >= VDELAY:
                    emit_v(sc - VDELAY)
                if sc % 4 == 3:
                    emit_qk(0, sc // 4)
                if sc == 7:
                    emit_group(0, 0)
                if sc == 11:
                    emit_group(0, 1)
            for sc in range(S // P - VDELAY, S // P):
                emit_v(sc)

            # ---- remaining attention groups (pair 0 groups 0-1 were emitted
            # inside the x loop); next pair's q/k interleaved ----
            for p2 in range(NP):
                for j in range(J):
                    if p2 == 0 and j < 2:
                        emit_qk(1, j)
                        continue
                    emit_group(p2, j)
                    if p2 + 1 < NP:
                        emit_qk(p2 + 1, j)
            flush_pending()

    nc.finalize()
    return nc


_NC_CACHE = {}


def _get_nc():
    key = "mha"
    if key not in _NC_CACHE:
        _NC_CACHE[key] = build_mha_nc()
    return _NC_CACHE[key]


def _arr_xT(xb, bf16):
    # [S, E] f32 -> [P, EC, S] bf16 with xT[p, ec, s] = x[s, ec*128+p]
    P, S, E = 128, xb.shape[0], xb.shape[1]
    xt = xb.astype(bf16).T.reshape(E // P, P, S)
    return np.ascontiguousarray(xt.transpose(1, 0, 2))


def _arr_wqk(w, bf16):
    # [H, E, D] -> [P, NP, EC, 2*D] pair-packed lhsT layout
    H, E, D = w.shape
    P = 128
    v = w.astype(bf16).reshape(H // 2, 2, E // P, P, D)
    v = v.transpose(3, 0, 2, 1, 4)  # [P, NP, EC, 2, D]
    return np.ascontiguousarray(v.reshape(P, H // 2, E // P, 2 * D))


def _arr_wv(w, bf16):
    # [H, E, D] -> [P, EC, H, D]
    H, E, D = w.shape
    P = 128
    v = w.astype(bf16).reshape(H, E // P, P, D)
    return np.ascontiguousarray(v.transpose(2, 1, 0, 3))


def _arr_wo(w, bf16):
    # [E, HD] -> [P, HC, E] with woT[p, hc, e] = Wo[e, hc*128+p]
    E, HD = w.shape
    P = 128
    v = w.astype(bf16).T.reshape(HD // P, P, E)
    return np.ascontiguousarray(v.transpose(1, 0, 2))


def kernel(x, Wq, Wk, Wv, Wo, bo, _runner_kwargs=None):
    import ml_dtypes
    bf16 = ml_dtypes.bfloat16
    x = np.asarray(x, dtype=np.float32)
    Wq = np.asarray(Wq, dtype=np.float32)
    Wk = np.asarray(Wk, dtype=np.float32)
    Wv = np.asarray(Wv, dtype=np.float32)
    Wo = np.asarray(Wo, dtype=np.float32)
    bo = np.asarray(bo, dtype=np.float32)

    HPC = NUM_HEADS // 2  # heads per core
    HDS = HPC * HEAD      # concat-dim slice per core

    nc = _get_nc()
    xbs = [_arr_xT(x[b], bf16) for b in range(BATCH)]
    in_maps = []
    for c in range(N_CORES):
        b, g = c // 2, c % 2
        hs = slice(g * HPC, (g + 1) * HPC)
        in_maps.append({
            "xT_in": xbs[b],
            "wq_in": _arr_wqk(Wq[hs], bf16),
            "wk_in": _arr_wqk(Wk[hs], bf16),
            "wv_in": _arr_wv(Wv[hs], bf16),
            "wo_in": _arr_wo(Wo[:, g * HDS:(g + 1) * HDS], bf16),
        })

    kw = dict(_runner_kwargs or {})
    res = bass_utils.run_bass_kernel_spmd(
        nc, in_maps, core_ids=list(range(N_CORES)), **kw
    )

    out = np.empty((BATCH, SEQ, EMB), dtype=np.float32)
    for b in range(BATCH):
        out[b] = res.results[2 * b]["out_p"] + res.results[2 * b + 1]["out_p"] + bo
    if kw.get("trace"):
        kernel.last_results = res
    return out


# revision 34
# speedup vs baseline: 1.2377x; 1.0022x over previous
"""Self-contained TRN2 Bass kernel for the 16-head MHA problem.

kernel(**inputs) takes FULL inputs (x [4,2048,1024], Wq/Wk/Wv [16,1024,64],
Wo [1024,1024], bo [1024]) and returns the FULL output [4,2048,1024] f32.

Sharding over 8 NeuronCores: core c handles batch b = c//2 and head group
g = c%2 (8 of 16 heads) — tensor parallel over heads with the output
projection's input dim sharded; the 2-way partial-sum reduce per batch and
the bias add happen host-side on the gathered results.
"""
import sys

for _p in ("/opt/trn_rl_repo",):
    if _p not in sys.path:
        sys.path.insert(0, _p)

import numpy as np
import concourse.bass as bass
import concourse.mybir as mybir
from concourse import bacc
from concourse.bass import ts, ds
from concourse.tile import TileContext
from concourse.vector_clock import ScopedClock
from concourse import bass_utils

F32 = mybir.dt.float32
BF16 = mybir.dt.bfloat16
AF = mybir.ActivationFunctionType

NUM_HEADS = 16
EMB = 1024
HEAD = 64
SEQ = 2048
BATCH = 4
N_CORES = 8


class TC(TileContext):
    """TileContext whose final drain splits its sem waits across SP NOPs —
    the CTRL instruction encoding holds only one wait and this env's Tile
    puts the whole global clock on the tail drain."""

    def _drain_and_barrier(self, tick_clock, wait_clock):
        nc = self.nc
        dummy = nc.sync.nop(nofuse=True)
        wait_clock.add_sem_waits(dummy.ins, ScopedClock({None: tick_clock.global_clock}))
        si = dummy.ins.sync_info
        waits = list(si.on_wait) if si is not None else []
        if len(waits) > 1:
            si.on_wait = waits[:1]
            sem_by_name = {h.name: h for h in self.sems.allocated().values()}
            for w in waits[1:]:
                nop = nc.sync.nop(nofuse=True)
                nop._wait_ge(sem_by_name[w.ant_name], w.wait_value)
        nc.sync.drain()
        nc.all_engine_barrier()
        popped = nc._tile_sem_poison_stack.pop()
        assert popped is self._sem_poison
        nc.clear_and_free_semaphores(list(self.sems.allocated().values()))
        nc.all_engine_barrier()


def build_mha_nc(S=SEQ, E=EMB, D=HEAD, H=NUM_HEADS // 2):
    """Single-core SPMD program; H = heads per core (pair-packed).

    Fully transposed formulation:
      xT (PE-transposed once), qT/kT per pair (q pre-scaled by 1/sqrt(D)),
      scoresT = kT-slice.T @ qT-slice (K=64 row groups, both heads of a
      pair into one 2-bank PSUM tile),
      attnT = exp(scoresT) (scores bounded, no max subtraction; one ACT op
      and one causal affine_select per pair-tile),
      ctxT+denominator = [v|1].T @ attnT accumulated over k chunks,
      reciprocal of the denominator via DMA-reshape to [128, x] so all DVE
      lanes work, broadcast along d via a K=1 ones matmul,
      out = concatT-chunk.T @ WoT-chunk accumulated over head-dim chunks.

    Scheduling structure: q/k projections for pair p+1 are interleaved
    into pair p's attention groups to fill PE gaps; ce accumulators are
    tagged by q-chunk parity so a group's normalization tail overlaps the
    next group's matmuls.
    """
    P = 128
    EC = E // P
    NQ = 512
    J = S // NQ
    KK = S // P
    NP = H // 2
    HD = H * D
    HC = HD // P
    NE = min(512, E)
    JE = E // NE
    RQ = NQ // 64          # reshape width for the two-row reciprocal trick

    nc = bacc.Bacc("TRN2", target_bir_lowering=False, debug=False)
    # host pre-casts x to bf16 and pre-arranges the (small) weights into
    # their on-chip layouts, halving the startup DMA bytes
    xT_in = nc.dram_tensor("xT_in", [P, EC, S], BF16, kind="ExternalInput")
    wq_in = nc.dram_tensor("wq_in", [P, NP, EC, 2 * D], BF16, kind="ExternalInput")
    wk_in = nc.dram_tensor("wk_in", [P, NP, EC, 2 * D], BF16, kind="ExternalInput")
    wv_in = nc.dram_tensor("wv_in", [P, EC, H, D], BF16, kind="ExternalInput")
    wo_in = nc.dram_tensor("wo_in", [P, HC, E], BF16, kind="ExternalInput")
    out_p = nc.dram_tensor("out_p", [S, E], F32, kind="ExternalOutput")

    scale = D ** -0.5

    with TC(nc) as tc:
        with (
            tc.tile_pool(name="const", bufs=1) as cpool,
            tc.tile_pool(name="persist", bufs=1) as pers,
            tc.tile_pool(name="stage", bufs=5) as stg,
            tc.tile_pool(name="attn", bufs=6) as apool,
            tc.tile_pool(name="small", bufs=3) as spool,
            tc.tile_pool(name="psS", bufs=4, space="PSUM") as psS,
            tc.tile_pool(name="psC", bufs=1, space="PSUM") as psC,
        ):
            ones_col = cpool.tile([1, D], mybir.dt.float16, tag="ones")
            nc.gpsimd.memset(ones_col[:], 1.0)
            warm_row = cpool.tile([1, NQ], mybir.dt.float16, tag="warmrow")
            nc.gpsimd.memset(warm_row[:], 0.0)

            xT = pers.tile([P, EC, S], BF16, tag="xT")
            qT = pers.tile([P, NP, S], BF16, tag="qT")
            kT = pers.tile([P, NP, S], BF16, tag="kT")
            v_pad = pers.tile([P, KK, H, D + 1], BF16, tag="vp")
            woT = pers.tile([P, HC, E], BF16, tag="woT")
            concatT = pers.tile([P, NP, S], BF16, tag="concT")
            wq_bf = pers.tile([P, NP, EC, 2 * D], BF16, tag="wq")
            wk_bf = pers.tile([P, NP, EC, 2 * D], BF16, tag="wk")
            wv_bf = pers.tile([P, EC, H, D], BF16, tag="wv")

            nc.gpsimd.memset(v_pad[:, :, :, D:D + 1], 1.0)

            # ---- Phase A: stream host-transposed bf16 xT in fine slices,
            # with v and pair-0 q/k projections fused in; weights land via
            # single pre-arranged DMAs queued behind the first slices ----
            # HAM warmup: keep the PE busy on throwaway matmuls while the
            # first DMAs stream, so real matmuls start at full clock
            warm_ps = psS.tile([D, NQ], F32, tag="sc2", name="warm_ps")
            for _w in range(12):
                nc.tensor.matmul(warm_ps[:], ones_col[:], warm_row[:],
                                 start=True, stop=True)

            nc.sync.dma_start(xT[:, :, ts(0, P)], xT_in[:, :, ts(0, P)])
            nc.sync.dma_start(wv_bf[:], wv_in[:])
            for sc in range(1, 4):
                nc.sync.dma_start(xT[:, :, ts(sc, P)], xT_in[:, :, ts(sc, P)])
            nc.sync.dma_start(wq_bf[:], wq_in[:])
            nc.sync.dma_start(wk_bf[:], wk_in[:])

            def emit_qk(p2, sc):
                # sc indexes NQ-wide chunks (two per call site index)
                for w_sb, dst, sc_mul in ((wq_bf, qT, scale), (wk_bf, kT, 1.0)):
                    acc = psS.tile(
                        [P, NQ], F32, tag="sc2",
                        name=f"qk_{p2}_{sc}_{0 if dst is qT else 1}",
                    )
                    for ec in range(EC):
                        nc.tensor.matmul(
                            acc[:],
                            w_sb[:, p2, ec, :],
                            xT[:, ec, ts(sc, NQ)],
                            start=(ec == 0), stop=(ec == EC - 1),
                        )
                    if sc_mul != 1.0:
                        nc.vector.tensor_scalar_mul(
                            dst[:, p2, ts(sc, NQ)], acc[:], sc_mul
                        )
                    else:
                        nc.vector.tensor_copy(dst[:, p2, ts(sc, NQ)], acc[:])

            def emit_v(sc):
                acc = psS.tile([P, HD], F32, tag="sc2", name=f"vacc_{sc}")
                for ec in range(EC):
                    nc.tensor.matmul(
                        acc[:],
                        xT[:, ec, ts(sc, P)],
                        wv_bf[:, ec, :, :].rearrange("p h d -> p (h d)"),
                        start=(ec == 0), stop=(ec == EC - 1),
                    )
                nc.vector.tensor_copy(
                    v_pad[:, sc, :, 0:D],
                    acc[:].rearrange("p (h d) -> p h d", d=D),
                )

            def emit_out_chunks(j):
                for sc in range(4 * j, 4 * j + 4):
                    for n in range(JE):
                        acc = psS.tile([P, NE], F32, tag="sc2",
                                       name=f"oacc_{sc}_{n}")
                        for hc in range(HC):
                            nc.tensor.matmul(
                                acc[:],
                                concatT[:, hc, ts(sc, P)],
                                woT[:, hc, ts(n, NE)],
                                start=(hc == 0), stop=(hc == HC - 1),
                            )
                        ot = stg.tile([P, NE], F32, tag="ostg")
                        nc.vector.tensor_copy(ot[:], acc[:])
                        nc.sync.dma_start(out_p[ts(sc, P), ts(n, NE)], ot[:])

            def emit_normalize(p2, j, ce):
                dens = [
                    spool.tile([1, NQ], F32, tag=f"den{hh}",
                               name=f"den{hh}_{p2}_{j}")
                    for hh in range(2)
                ]
                for hh in range(2):
                    nc.vector.tensor_copy(dens[hh][:], ce[hh][ds(D, 1), :])
                den_rs = spool.tile([P, RQ], F32, tag="denrs")
                for hh in range(2):
                    nc.sync.dma_start(
                        den_rs[ds(hh * (P // 2), P // 2), :], dens[hh][:]
                    )
                rc = spool.tile([P, RQ], F32, tag="rc")
                nc.vector.reciprocal(rc[:], den_rs[:])
                rch = spool.tile([P, RQ], mybir.dt.float16, tag="rch")
                nc.vector.tensor_copy(rch[:], rc[:])
                recips = [
                    spool.tile([1, NQ], mybir.dt.float16, tag=f"recip{hh}",
                               name=f"recip{hh}_{p2}_{j}")
                    for hh in range(2)
                ]
                for hh in range(2):
                    nc.sync.dma_start(
                        recips[hh][:], rch[ds(hh * (P // 2), P // 2), :]
                    )
                for hh in range(2):
                    bc = psS.tile([D, NQ], F32, tag="sc2", name=f"bc_{p2}_{j}_{hh}")
                    nc.tensor.matmul(
                        bc[:], ones_col[:], recips[hh][:],
                        start=True, stop=True,
                    )
                    bc_sb = spool.tile([D, NQ], F32, tag="bcsb")
                    nc.vector.tensor_copy(bc_sb[:], bc[:])
                    nc.vector.tensor_mul(
                        concatT[ds(hh * D, D), p2, ts(j, NQ)],
                        ce[hh][0:D, :], bc_sb[:],
                    )

            pending = [None]

            def flush_pending():
                if pending[0] is not None:
                    p2x, jx, cex = pending[0]
                    emit_normalize(p2x, jx, cex)
                    if p2x == NP - 1:
                        emit_out_chunks(jx)
                    pending[0] = None

            def emit_group(p2, j):
                h0, h1 = 2 * p2, 2 * p2 + 1
                n_kk = min(KK, 4 * j + 4)
                ce = [
                    psC.tile([D + 1, NQ], F32, tag=f"ce{hh}{j % 2}",
                             name=f"ce{hh}_{p2}_{j}")
                    for hh in range(2)
                ]
                for i in range(n_kk):
                    t = i - 4 * j  # >= 0 -> diagonal (partial) tile
                    q0 = P * t if t > 0 else 0
                    nq = NQ - q0
                    for hh, hloc in enumerate((h0, h1)):
                        sps = psS.tile([P, NQ], F32, tag="sc2",
                                       name=f"s2_{p2}_{j}_{i}_{hh}")
                        nc.tensor.matmul(
                            sps[:, 0:nq],
                            kT[ds(hh * D, D), p2, ts(i, P)],
                            qT[ds(hh * D, D), p2, ds(j * NQ + q0, nq)],
                            start=True, stop=True,
                        )
                        at = apool.tile([P, NQ], BF16, tag="at")
                        nc.scalar.activation(at[:, 0:nq], sps[:, 0:nq], AF.Exp)
                        if t >= 0:
                            nc.gpsimd.affine_select(
                                out=at[:, 0:nq], in_=at[:, 0:nq],
                                compare_op=mybir.AluOpType.is_ge,
                                fill=0.0, base=P * t - q0,
                                pattern=[[1, nq]], channel_multiplier=-1,
                            )
                        nc.tensor.matmul(
                            ce[hh][0:D + 1, ds(q0, nq)],
                            v_pad[:, i, hloc, :],
                            at[:, 0:nq],
                            start=(i == 0), stop=(i == n_kk - 1),
                        )
                    if i == min(3, n_kk - 1):
                        flush_pending()
                flush_pending()
                pending[0] = (p2, j, ce)

            VDELAY = 2
            for sc in range(S // P):
                if sc >= 4:
                    nc.sync.dma_start(xT[:, :, ts(sc, P)], xT_in[:, :, ts(sc, P)])
                if sc == 2:
                    nc.sync.dma_start(woT[:], wo_in[:])
                if sc >= VDELAY:
                    emit_v(sc - VDELAY)
                if sc % 4 == 3:
                    emit_qk(0, sc // 4)
                if sc == 7:
                    emit_group(0, 0)
                if sc == 11:
                    emit_group(0, 1)
            for sc in range(S // P - VDELAY, S // P):
                emit_v(sc)

            # ---- remaining attention groups (pair 0 groups 0-1 were emitted
            # inside the x loop); next pair's q/k interleaved ----
            for p2 in range(NP):
                for j in range(J):
                    if p2 == 0 and j < 2:
                        emit_qk(1, j)
                        continue
                    emit_group(p2, j)
                    if p2 + 1 < NP:
                        emit_qk(p2 + 1, j)
            flush_pending()

    nc.finalize()
    return nc


_NC_CACHE = {}


def _get_nc():
    key = "mha"
    if key not in _NC_CACHE:
        _NC_CACHE[key] = build_mha_nc()
    return _NC_CACHE[key]


def _arr_xT(xb, bf16):
    # [S, E] f32 -> [P, EC, S] bf16 with xT[p, ec, s] = x[s, ec*128+p]
    P, S, E = 128, xb.shape[0], xb.shape[1]
    xt = xb.astype(bf16).T.reshape(E // P, P, S)
    return np.ascontiguousarray(xt.transpose(1, 0, 2))


def _arr_wqk(w, bf16):
    # [H, E, D] -> [P, NP, EC, 2*D] pair-packed lhsT layout
    H, E, D = w.shape
    P = 128
    v = w.astype(bf16).reshape(H // 2, 2, E // P, P, D)
    v = v.transpose(3, 0, 2, 1, 4)  # [P, NP, EC, 2, D]
    return np.ascontiguousarray(v.reshape(P, H // 2, E // P, 2 * D))


def _arr_wv(w, bf16):
    # [H, E, D] -> [P, EC, H, D]
    H, E, D = w.shape
    P = 128
    v = w.astype(bf16).reshape(H, E // P, P, D)
    return np.ascontiguousarray(v.transpose(2, 1, 0, 3))


def _arr_wo(w, bf16):
    # [E, HD] -> [P, HC, E] with woT[p, hc, e] = Wo[e, hc*128+p]
    E, HD = w.shape
    P = 128
    v = w.astype(bf16).T.reshape(HD // P, P, E)
    return np.ascontiguousarray(v.transpose(1, 0, 2))


def kernel(x, Wq, Wk, Wv, Wo, bo, _runner_kwargs=None):
    import ml_dtypes
    bf16 = ml_dtypes.bfloat16
    x = np.asarray(x, dtype=np.float32)
    Wq = np.asarray(Wq, dtype=np.float32)
    Wk = np.asarray(Wk, dtype=np.float32)
    Wv = np.asarray(Wv, dtype=np.float32)
    Wo = np.asarray(Wo, dtype=np.float32)
    bo = np.asarray(bo, dtype=np.float32)

    HPC = NUM_HEADS // 2  # heads per core
    HDS = HPC * HEAD      # concat-dim slice per core

    nc = _get_nc()
    xbs = [_arr_xT(x[b], bf16) for b in range(BATCH)]
    in_maps = []
    for c in range(N_CORES):
        b, g = c // 2, c % 2
        hs = slice(g * HPC, (g + 1) * HPC)
        in_maps.append({
            "xT_in": xbs[b],
            "wq_in": _arr_wqk(Wq[hs], bf16),
            "wk_in": _arr_wqk(Wk[hs], bf16),
            "wv_in": _arr_wv(Wv[hs], bf16),
            "wo_in": _arr_wo(Wo[:, g * HDS:(g + 1) * HDS], bf16),
        })

    kw = dict(_runner_kwargs or {})
    res = bass_utils.run_bass_kernel_spmd(
        nc, in_maps, core_ids=list(range(N_CORES)), **kw
    )

    out = np.empty((BATCH, SEQ, EMB), dtype=np.float32)
    for b in range(BATCH):
        out[b] = res.results[2 * b]["out_p"] + res.results[2 * b + 1]["out_p"] + bo
    if kw.get("trace"):
        kernel.last_results = res
    return out
